# revision 29
# baseline (speedup 1.0000x reference)
"""Trainium2 Bass kernel for nn_Decoder (sparse_attention).

Data-parallel over batch: B=64 split across 8 NeuronCores (8 batch elems each).
Per core, the full decoder runs with a channel-major layout ([C, T] on-chip):

  fc1 -> conv1(GLU,res) -> attn1 -> conv2..4 -> attn4 -> fc2/fc3 heads

Key layout choices:
  - x is kept channel-major [256, T=512] (2 partition tiles) through the net;
    conv taps become plain matmuls over shifted time slices of a left-padded
    SBUF buffer, attention Q/out projections read/write the same layout.
  - attn folds Wq/Wk into Wqk = Wq^T@Wk (bk drops out of softmax) and
    Wv/Wo into Wov = Wo@Wv (bv folds into the output bias via softmax sum=1),
    so keys/values are used raw: scores = (x^T Wqk) keys^T, out = Wov(attn V).
  - keys^T is produced on-chip via PE transposes (fp32r); attn^T likewise
    (bf16) to feed the attn@V matmul, which uses values in natural layout.
  - softmax skips max-subtraction (scores are in [-3, 3]) and gets row sums
    for free from the ACT Exp accum_out port.
  - dtypes: fp32r (tf32-grade, full PE speed) for the score path,
    bf16 for conv/projection weights and the x chain (SBUF capacity),
    f32 accumulation everywhere (PSUM).

Scheduling (Tile executes each engine's stream strictly in order, so emission
order is the schedule): batch elements are emitted in interleaved pairs so the
partner's matmuls cover every serial ACT/DVE tail (conv GLU, softmax);
attention is emitted in four sub-phases (q2 / scores+softmax / attn-transpose
/ A+proj) alternating between the pair so no PE instruction ever queues
behind a stalled one (head-of-line blocking); the attn@V accumulation is
st-major across all four output psums so PE consumption tracks the DVE
attn^T-evacuation rate; each pair's fc1+keys stages are hoisted before the
previous pair's output stage; weights load on the sync ring split
early/mid/late around the first input loads; outputs/aligns go out via the
otherwise-idle GpSimd SWDGE ring.

Measured on 8 axon-tunneled NeuronCores: ~733 us HW exec (728-738 over runs),
worst output rel-err 6.9e-3 (align1), PE ~96% busy within the span.
"""

import math
import numpy as np
import ml_dtypes

import concourse.bacc as bacc
import concourse.bass as bass
import concourse.tile as tile
import concourse.mybir as mybir
from concourse import bass_utils

F32 = mybir.dt.float32
F32R = mybir.dt.float32r
BF16 = mybir.dt.bfloat16
AF = mybir.ActivationFunctionType
ALU = mybir.AluOpType
BF16NP = ml_dtypes.bfloat16

NCORES = 8
B, T, S = 64, 512, 1024
PB = B // NCORES            # batch elems per core
DD, DE = 256, 512           # decoder dim / encoder dim
INCH = 320                  # fc1 in, fc2 out
KW = 5                      # conv kernel width
DILS = [1, 2, 2, 3]
PADS = [(KW - 1) * d for d in DILS]   # 4, 8, 8, 12
C = math.sqrt(0.5)
SQS = math.sqrt(S)

_cached = {}


# --------------------------------------------------------------------------
# host-side weight packing
# --------------------------------------------------------------------------

def _np(x):
    return np.asarray(x, dtype=np.float32)


def _pack_weights(params):
    p = params
    w = {}
    w["w1t"] = _np(p["fc1_W"]).T.copy().astype(BF16NP)            # [320, 256]

    convw = np.zeros((4, 2, 128, KW, 2 * DD), dtype=np.float32)
    for i in range(4):
        Wc = _np(p["convs"][i][0])                                 # [512, 256, 5]
        Ws = np.concatenate([Wc[:DD] * C, Wc[DD:]], axis=0)        # scale a-half
        convw[i] = Ws.transpose(1, 2, 0).reshape(2, 128, KW, 2 * DD)
    w["convw"] = convw.astype(BF16NP)

    wqk = np.zeros((2, 2, 128, DE), dtype=np.float32)
    wovt = np.zeros((2, 4, 128, DD), dtype=np.float32)
    bqk = np.zeros((2, DE), dtype=np.float32)
    bov = np.zeros((2, DD), dtype=np.float32)
    for j, key in enumerate(("attn1", "attn4")):
        ap = p[key]
        Wq, bq = _np(ap["Wq"]), _np(ap["bq"])
        Wk = _np(ap["Wk"])
        Wv, bv = _np(ap["Wv"]), _np(ap["bv"])
        Wo, bo = _np(ap["Wo"]), _np(ap["bo"])
        wqk[j] = (Wq.T @ Wk).reshape(2, 128, DE)
        bqk[j] = bq @ Wk
        wov = (C * SQS) * (Wo @ Wv)                                # [256, 512]
        wovt[j] = wov.T.reshape(4, 128, DD)
        bov[j] = C * (SQS * (Wo @ bv) + bo)
    w["wqk"] = wqk.astype(BF16NP)
    w["wovt"] = wovt.astype(BF16NP)

    w23 = np.concatenate([_np(p["fc2_W"]), _np(p["fc3_W"])], axis=0)  # [321, 256]
    w["w23t"] = w23.T.reshape(2, 128, 321).astype(BF16NP)
    w["b23"] = np.concatenate([_np(p["fc2_b"]), _np(p["fc3_b"])])[None, :] \
        .astype(BF16NP)                                            # [1, 321]

    # per-partition biases, packed [128, n] column-per-bias
    cols = []
    b1 = _np(p["fc1_b"])
    cols += [b1[:128], b1[128:]]                                   # 0..1
    for i in range(4):
        bc = _np(p["convs"][i][1])
        cols += [bc[:128] * C, bc[128:256] * C]                    # a-half (scaled)
        cols += [bc[256:384], bc[384:]]                            # g-half
    for j in range(2):                                             # 18..25
        for d4 in range(4):
            cols.append(bqk[j, d4 * 128:(d4 + 1) * 128])
    for j in range(2):                                             # 26..29
        for ct in range(2):
            cols.append(bov[j, ct * 128:(ct + 1) * 128])
    w["biases"] = np.stack(cols, axis=1).astype(np.float32)        # [128, 30]
    return w

BIAS_B1 = 0
def _bias_conv_a(i, h): return 2 + i * 4 + h
def _bias_conv_g(i, h): return 2 + i * 4 + 2 + h
def _bias_qk(j, d4): return 18 + j * 4 + d4
def _bias_ov(j, ct): return 26 + j * 2 + ct


# --------------------------------------------------------------------------
# device program
# --------------------------------------------------------------------------

def _build():
    nc = bacc.Bacc("TRN2", target_bir_lowering=False, debug=False,
                   enable_asserts=False, num_devices=NCORES)

    d_xin = nc.dram_tensor("xin", [PB, T, INCH], BF16, kind="ExternalInput")
    d_keys = nc.dram_tensor("keys", [PB, S, DE], F32R, kind="ExternalInput")
    d_vals = nc.dram_tensor("vals", [PB, S, DE], BF16, kind="ExternalInput")
    d_w1t = nc.dram_tensor("w1t", [INCH, DD], BF16, kind="ExternalInput")
    d_convw = nc.dram_tensor("convw", [4, 2, 128, KW, 2 * DD], BF16, kind="ExternalInput")
    d_wqk = nc.dram_tensor("wqk", [2, 2, 128, DE], BF16, kind="ExternalInput")
    d_wovt = nc.dram_tensor("wovt", [2, 4, 128, DD], BF16, kind="ExternalInput")
    d_w23t = nc.dram_tensor("w23t", [2, 128, 321], BF16, kind="ExternalInput")
    d_b23 = nc.dram_tensor("b23", [1, 321], BF16, kind="ExternalInput")
    d_biases = nc.dram_tensor("biases", [128, 30], F32, kind="ExternalInput")
    d_identr = nc.dram_tensor("identr", [128, 128], F32R, kind="ExternalInput")
    d_identf = nc.dram_tensor("identf", [128, 128], F32, kind="ExternalInput")
    d_identb = nc.dram_tensor("identb", [128, 128], BF16, kind="ExternalInput")

    d_out = nc.dram_tensor("out", [PB, T, INCH], F32, kind="ExternalOutput")
    d_states = nc.dram_tensor("states", [PB, T, DD], F32, kind="ExternalOutput")
    d_done = nc.dram_tensor("done", [PB, T], F32, kind="ExternalOutput")
    d_align = [nc.dram_tensor("align1", [PB, T, S], BF16, kind="ExternalOutput"),
               nc.dram_tensor("align4", [PB, T, S], BF16, kind="ExternalOutput")]

    with tile.TileContext(nc) as tc:
        _emit(nc, tc, d_xin, d_keys, d_vals, d_w1t, d_convw, d_wqk, d_wovt,
              d_w23t, d_b23, d_biases, d_identr, d_identf, d_identb,
              d_out, d_states, d_done, d_align)
    nc.compile()
    return nc


def _emit(nc, tc, d_xin, d_keys, d_vals, d_w1t, d_convw, d_wqk, d_wovt,
          d_w23t, d_b23, d_biases, d_identr, d_identf, d_identb,
          d_out, d_states, d_done, d_align):
    from contextlib import ExitStack
    ctx = ExitStack()
    wp = ctx.enter_context(tc.tile_pool(name="weights", bufs=1))
    xp = ctx.enter_context(tc.tile_pool(name="xchain", bufs=2))
    kp = ctx.enter_context(tc.tile_pool(name="kv", bufs=2))
    ap_ = ctx.enter_context(tc.tile_pool(name="attn", bufs=2))
    sp = ctx.enter_context(tc.tile_pool(name="small", bufs=2))
    op = ctx.enter_context(tc.tile_pool(name="outs", bufs=2))
    ps = ctx.enter_context(tc.tile_pool(name="ps", bufs=6, space="PSUM"))
    pt = ctx.enter_context(tc.tile_pool(name="pstr", bufs=2, space="PSUM"))

    # ---- pair-0 inputs first on the sync ring (first matmul gates on them),
    # then the critical weights; the bulk (convw[1:], attn4/fc2 weights) is
    # emitted after pair 0's keys loads, arriving well before first use -----
    pre_inA = {}
    for b in (0, 1):
        t_ = sp.tile([128, 4 * INCH], BF16, tag="inA", name=f"inA{b}")
        nc.sync.dma_start(
            t_[:], d_xin.ap()[b].rearrange("(tt tp) c -> tp tt c", tp=128))
        pre_inA[b] = t_
    identb = wp.tile([128, 128], BF16, tag="identb", name="identb")
    nc.sync.dma_start(identb[:], d_identb.ap())
    w1t_sb = []
    for kk in range(3):
        kw_ = 64 if kk == 2 else 128
        t_ = wp.tile([kw_, DD], BF16, tag=f"w1t{kk}", name=f"w1t{kk}")
        nc.sync.dma_start(t_[:], d_w1t.ap()[kk * 128:kk * 128 + kw_, :])
        w1t_sb.append(t_)
    bias_sb = wp.tile([128, 30], F32, tag="bias", name="bias")
    nc.sync.dma_start(bias_sb[:], d_biases.ap())
    identr = wp.tile([128, 128], F32R, tag="identr", name="identr")
    convw_sb = [[wp.tile([128, KW * 2 * DD], BF16, tag=f"cw{i}{ict}",
                         name=f"cw{i}{ict}")
                 for ict in range(2)] for i in range(4)]
    wqk_sb = [[wp.tile([128, DE], BF16, tag=f"wqk{j}{ct}", name=f"wqk{j}{ct}")
               for ct in range(2)] for j in range(2)]
    wovt_sb = [[wp.tile([128, DD], BF16, tag=f"wov{j}{d4}", name=f"wov{j}{d4}")
                for d4 in range(4)] for j in range(2)]
    w23t_sb = [wp.tile([128, 321], BF16, tag=f"w23{ct}", name=f"w23{ct}")
               for ct in range(2)]
    b23_sb = wp.tile([1, 321], BF16, tag="b23", name="b23")
    identf = wp.tile([128, 128], F32, tag="identf", name="identf")
    onesb = wp.tile([1, 128], BF16, tag="ones", name="ones")
    nc.vector.memset(onesb[:], 1.0)

    def load_mid_weights():
        nc.sync.dma_start(identr[:], d_identr.ap())
        for ict in range(2):
            nc.sync.dma_start(convw_sb[0][ict][:], d_convw.ap()[0, ict])
        for ct in range(2):
            nc.sync.dma_start(wqk_sb[0][ct][:], d_wqk.ap()[0, ct])
        for d4 in range(4):
            nc.sync.dma_start(wovt_sb[0][d4][:], d_wovt.ap()[0, d4])

    def load_late_weights():
        for i in range(1, 4):
            for ict in range(2):
                nc.sync.dma_start(convw_sb[i][ict][:], d_convw.ap()[i, ict])
        for ct in range(2):
            nc.sync.dma_start(wqk_sb[1][ct][:], d_wqk.ap()[1, ct])
        for d4 in range(4):
            nc.sync.dma_start(wovt_sb[1][d4][:], d_wovt.ap()[1, d4])
        for ct in range(2):
            nc.sync.dma_start(w23t_sb[ct][:], d_w23t.ap()[ct])
        nc.sync.dma_start(b23_sb[:], d_b23.ap())
        nc.sync.dma_start(identf[:], d_identf.ap())

    def bias(i):
        return bias_sb[:, i:i + 1]

    # ---- stages -------------------------------------------------------------
    def load_inA(b):
        inA = sp.tile([128, 4 * INCH], BF16, tag="inA", name=f"inA{b}")
        nc.sync.dma_start(
            inA[:], d_xin.ap()[b].rearrange("(tt tp) c -> tp tt c", tp=128))
        return inA

    def fc1_stage(b, xp1, inA):
        inT = []
        for cc in range(3):
            cw = 64 if cc == 2 else 128
            pmt = pt.tile([128, T], BF16, tag="tr", name=f"inT{b}{cc}")
            for tt in range(4):
                nc.tensor.matmul(
                    pmt[:cw, tt * 128:tt * 128 + 128],
                    inA[:, tt * INCH + cc * 128:tt * INCH + cc * 128 + cw],
                    identb[:], is_transpose=True)
            it = sp.tile([128, T], BF16, tag=f"inT{cc}", name=f"inTs{b}{cc}")
            nc.vector.tensor_copy(it[:cw, :], pmt[:cw, :])
            inT.append(it)
        for ct in range(2):
            pm = ps.tile([128, T], F32, tag="mm", name=f"fc1{b}{ct}")
            for kk in range(3):
                kw_ = 64 if kk == 2 else 128
                nc.tensor.matmul(pm[:],
                                 w1t_sb[kk][:kw_, ct * 128:ct * 128 + 128],
                                 inT[kk][:kw_, :],
                                 start=(kk == 0), stop=(kk == 2))
            nc.vector.memset(xp1[ct][:, 0:PADS[0]], 0.0)
            nc.scalar.activation(xp1[ct][:, PADS[0]:PADS[0] + T], pm[:], AF.Relu,
                                 bias=bias(BIAS_B1 + ct))
        return xp1

    def kv_stage(b):
        keysT = [kp.tile([128, S], F32R, tag=f"keysT{d4}", name=f"kT{b}{d4}")
                 for d4 in range(4)]
        krr = d_keys.ap()[b].rearrange("(st sp) d -> sp st d", sp=128)
        for half in range(2):
            kn2 = []
            for q in range(2):
                knat = kp.tile([128, 2 * DE], F32R, tag="knat",
                               name=f"kn{b}{half}{q}", bufs=2)
                nc.sync.dma_start(
                    knat[:], krr[:, half * 4 + q * 2:half * 4 + q * 2 + 2, :])
                kn2.append(knat)
            for d4 in range(4):
                pmt = pt.tile([128, 512], F32R, tag="tr",
                              name=f"kt{b}{half}{d4}")
                for l in range(4):
                    nc.tensor.matmul(pmt[:, l * 128:l * 128 + 128],
                                     kn2[l // 2][:, (l % 2) * DE + d4 * 128:
                                                 (l % 2) * DE + d4 * 128 + 128],
                                     identr[:], is_transpose=True)
                nc.scalar.copy(keysT[d4][:, half * 512:half * 512 + 512],
                               pmt[:])
        vnat = kp.tile([128, 8 * DE], BF16, tag="vnat", name=f"vn{b}")
        nc.sync.dma_start(
            vnat[:], d_vals.ap()[b].rearrange("(st sp) d -> sp st d", sp=128))
        return keysT, vnat

    def conv_block(i, b, src, soff, dst, doff):
        d = DILS[i]
        pms = {}
        for oct in (2, 3, 0, 1):          # g-half first: sigmoids overlap a-half
            pm = ps.tile([128, T], F32, tag="mm", name=f"cv{b}_{i}o{oct}")
            n = 0
            for ict in range(2):
                for k in range(KW):
                    nc.tensor.matmul(
                        pm[:],
                        convw_sb[i][ict][:, k * 2 * DD + oct * 128:
                                         k * 2 * DD + oct * 128 + 128],
                        src[ict][:, k * d:k * d + T],
                        start=(n == 0), stop=(n == 9))
                    n += 1
            pms[oct] = pm
        for h in range(2):
            sig = sp.tile([128, T], BF16, tag=f"sig{h}", name=f"cv{b}_{i}s{h}")
            nc.scalar.activation(sig[:], pms[2 + h][:], AF.Sigmoid,
                                 bias=bias(_bias_conv_g(i, h)))
            glu = sp.tile([128, T], BF16, tag=f"glu{h}", name=f"cv{b}_{i}g{h}")
            nc.vector.scalar_tensor_tensor(
                glu[:], pms[h][:], bias(_bias_conv_a(i, h)), sig[:],
                ALU.add, ALU.mult)
            nc.vector.scalar_tensor_tensor(
                dst[h][:, doff:doff + T], src[h][:, soff:soff + T], C, glu[:],
                ALU.mult, ALU.add)

    def attn_q2(j, b, src, soff):
        q2T = []
        for d4 in range(4):
            pm = ps.tile([128, T], F32, tag="mm", name=f"a{b}_{j}q{d4}")
            for ct in range(2):
                nc.tensor.matmul(pm[:],
                                 wqk_sb[j][ct][:, d4 * 128:d4 * 128 + 128],
                                 src[ct][:, soff:soff + T],
                                 start=(ct == 0), stop=(ct == 1))
            qt = ap_.tile([128, T], F32R, tag=f"q2T{d4}", name=f"a{b}_{j}qt{d4}")
            nc.scalar.activation(qt[:], pm[:], AF.Identity,
                                 bias=bias(_bias_qk(j, d4)))
            q2T.append(qt)
        return q2T

    def attn_scores(j, b, q2T, keysT):
        attnb = []
        for tt in range(4):
            eb = ap_.tile([128, S], BF16, tag=f"exp{tt}", name=f"a{b}_{j}e{tt}")
            zp = sp.tile([128, 2], F32, tag="zp", name=f"a{b}_{j}zp{tt}")
            for sh in range(2):
                pm = ps.tile([128, 512], F32, tag="mm", name=f"a{b}_{j}s{tt}{sh}")
                for d4 in range(4):
                    nc.tensor.matmul(pm[:],
                                     q2T[d4][:, tt * 128:tt * 128 + 128],
                                     keysT[d4][:, sh * 512:sh * 512 + 512],
                                     start=(d4 == 0), stop=(d4 == 3))
                nc.scalar.activation(eb[:, sh * 512:sh * 512 + 512], pm[:],
                                     AF.Exp, accum_out=zp[:, sh:sh + 1])
            z = sp.tile([128, 1], F32, tag="z", name=f"a{b}_{j}z{tt}")
            nc.vector.tensor_add(z[:], zp[:, 0:1], zp[:, 1:2])
            rec = sp.tile([128, 1], F32, tag="rec", name=f"a{b}_{j}r{tt}")
            nc.vector.reciprocal(rec[:], z[:])
            nc.vector.tensor_scalar_mul(eb[:], eb[:], rec[:])
            nc.gpsimd.dma_start(d_align[j].ap()[b, tt * 128:tt * 128 + 128, :],
                                eb[:])
            attnb.append(eb)
        return attnb

    def attn_tr(j, b, attnb):
        attnT = []
        for sg in range(4):
            pmt = pt.tile([128, 2 * T], BF16, tag="tr", name=f"a{b}_{j}t{sg}")
            for l in range(2):
                st = 2 * sg + l
                for tt in range(4):
                    nc.tensor.matmul(
                        pmt[:, l * T + tt * 128:l * T + tt * 128 + 128],
                        attnb[tt][:, st * 128:st * 128 + 128],
                        identb[:], is_transpose=True)
            at = ap_.tile([128, 2 * T], BF16, tag=f"attnT{sg}",
                          name=f"a{b}_{j}at{sg}")
            nc.vector.tensor_copy(at[:], pmt[:])
            attnT.append(at)
        return attnT

    def attn_av(j, b, attnT, vnat, src, soff, dst, doff):
        pmA = [ps.tile([128, T], F32, tag="mm", name=f"a{b}_{j}A{d4}")
               for d4 in range(4)]
        for st in range(8):
            for d4 in range(4):
                nc.tensor.matmul(pmA[d4][:],
                                 vnat[:, st * DE + d4 * 128:
                                      st * DE + d4 * 128 + 128],
                                 attnT[st // 2][:, (st % 2) * T:
                                                (st % 2) * T + T],
                                 start=(st == 0), stop=(st == 7))
        Asb = []
        for d4 in range(4):
            asb = ap_.tile([128, T], BF16, tag=f"Asb{d4}", name=f"a{b}_{j}as{d4}")
            nc.scalar.copy(asb[:], pmA[d4][:])
            Asb.append(asb)
        for ct in range(2):
            pm = ps.tile([128, T], F32, tag="mm", name=f"a{b}_{j}p{ct}")
            for d4 in range(4):
                nc.tensor.matmul(pm[:],
                                 wovt_sb[j][d4][:, ct * 128:ct * 128 + 128],
                                 Asb[d4][:],
                                 start=(d4 == 0), stop=(d4 == 3))
            tmp = sp.tile([128, T], BF16, tag=f"ptmp{ct}", name=f"a{b}_{j}pt{ct}")
            nc.scalar.activation(tmp[:], pm[:], AF.Identity,
                                 bias=bias(_bias_ov(j, ct)))
            nc.vector.scalar_tensor_tensor(
                dst[ct][:, doff:doff + T], src[ct][:, soff:soff + T], C,
                tmp[:], ALU.mult, ALU.add)

    def out_stage(b, scm):
        for tt in range(4):
            pmt = pt.tile([128, DD], BF16, tag="tr", name=f"st{b}{tt}")
            for ct in range(2):
                nc.tensor.matmul(pmt[:, ct * 128:ct * 128 + 128],
                                 scm[ct][:, tt * 128:tt * 128 + 128],
                                 identb[:], is_transpose=True)
            stg = op.tile([128, DD], F32, tag="st_sb", name=f"stsb{b}{tt}")
            nc.scalar.copy(stg[:], pmt[:])
            nc.gpsimd.dma_start(
                d_states.ap()[b, tt * 128:tt * 128 + 128, :], stg[:])
        dcol = op.tile([128, 4], F32, tag="dcol", name=f"dcol{b}")
        for tt in range(4):
            pm = ps.tile([128, 321], F32, tag="mm", name=f"fc2{b}{tt}")
            for ct in range(2):
                nc.tensor.matmul(pm[:],
                                 scm[ct][:, tt * 128:tt * 128 + 128],
                                 w23t_sb[ct][:],
                                 start=(ct == 0), stop=False)
            nc.tensor.matmul(pm[:], onesb[:, 0:128],
                             b23_sb[:], start=False, stop=True)
            nc.vector.tensor_copy(dcol[:, tt:tt + 1], pm[:, 320:321])
            osb = op.tile([128, 321], F32, tag="osb", name=f"osb{b}{tt}")
            nc.scalar.activation(osb[:], pm[:], AF.Sigmoid)
            nc.gpsimd.dma_start(
                d_out.ap()[b, tt * 128:tt * 128 + 128, :], osb[:, 0:INCH])
        pmt = pt.tile([128, 128], F32, tag="tr", name=f"dn{b}")
        nc.tensor.matmul(pmt[:4, :], dcol[:], identf[:], is_transpose=True)
        done_sb = op.tile([4, 128], F32, tag="done", name=f"dnsb{b}")
        nc.scalar.activation(done_sb[:], pmt[:4, :], AF.Sigmoid)
        nc.gpsimd.dma_start(
            d_done.ap()[b].rearrange("(n p) -> n p", n=4), done_sb[:])

    # ---- interleaved batch-element pairs: the partner's matmuls fill each
    # stage's serial ACT/DVE tail so the PE never drains ----------------------
    def xtile(side, b, k):
        return [xp.tile([128, 524], BF16, tag=f"xs{side}{ct}",
                        name=f"xs{side}_{b}{k}{ct}") for ct in range(2)]

    pend_out = []          # (b, scm) of the previous pair, not yet emitted
    for pair in range(PB // 2):
        bs = (2 * pair, 2 * pair + 1)
        st = {}
        ina = {b: (pre_inA[b] if pair == 0 else load_inA(b)) for b in bs}
        for b in bs:
            xp1 = xtile("A", b, 1)
            st[b] = {"xp1": fc1_stage(b, xp1, ina[b])}
        if pair == 0:
            load_mid_weights()
        for b in bs:
            st[b]["kv"] = kv_stage(b)
        if pair == 0:
            load_late_weights()
        for b, scm in pend_out:
            out_stage(b, scm)
        pend_out = []
        for b in bs:
            x = st[b]
            x["xc1"] = xtile("B", b, 1)
            conv_block(0, b, x["xp1"], PADS[0], x["xc1"], 0)
        for b in bs:
            x = st[b]
            x["q1"] = attn_q2(0, b, x["xc1"], 0)
        for b in bs:
            x = st[b]
            x["ab1"] = attn_scores(0, b, x["q1"], x["kv"][0])
        for b in bs:
            x = st[b]
            x["at1"] = attn_tr(0, b, x["ab1"])
        for b in bs:
            x = st[b]
            x["xp2"] = xtile("A", b, 2)
            for ct in range(2):
                nc.vector.memset(x["xp2"][ct][:, 0:PADS[1]], 0.0)
            keysT, vnat = x["kv"]
            attn_av(0, b, x["at1"], vnat, x["xc1"], 0, x["xp2"], PADS[1])
        for b in bs:
            x = st[b]
            x["xp3"] = xtile("B", b, 2)
            for ct in range(2):
                nc.vector.memset(x["xp3"][ct][:, 0:PADS[2]], 0.0)
            conv_block(1, b, x["xp2"], PADS[1], x["xp3"], PADS[2])
        for b in bs:
            x = st[b]
            x["xp4"] = xtile("A", b, 3)
            for ct in range(2):
                nc.vector.memset(x["xp4"][ct][:, 0:PADS[3]], 0.0)
            conv_block(2, b, x["xp3"], PADS[2], x["xp4"], PADS[3])
        for b in bs:
            x = st[b]
            x["xc4"] = xtile("B", b, 3)
            conv_block(3, b, x["xp4"], PADS[3], x["xc4"], 0)
        for b in bs:
            x = st[b]
            x["q4"] = attn_q2(1, b, x["xc4"], 0)
        for b in bs:
            x = st[b]
            x["ab4"] = attn_scores(1, b, x["q4"], x["kv"][0])
        for b in bs:
            x = st[b]
            x["at4"] = attn_tr(1, b, x["ab4"])
        for b in bs:
            x = st[b]
            x["scm"] = [xp.tile([128, T], BF16, tag=f"scm{ct}",
                                name=f"scm{b}{ct}") for ct in range(2)]
            keysT, vnat = x["kv"]
            attn_av(1, b, x["at4"], vnat, x["xc4"], 0, x["scm"], 0)
            pend_out.append((b, x["scm"]))
    for b, scm in pend_out:
        out_stage(b, scm)

    ctx.close()


# --------------------------------------------------------------------------
# entry point
# --------------------------------------------------------------------------

def _get_program():
    if "nc" not in _cached:
        _cached["nc"] = _build()
    return _cached["nc"]


def run(inputs, keys, values, params, trace=False):
    nc = _get_program()
    w = _pack_weights(params)
    xin = np.asarray(inputs, dtype=np.float32).astype(BF16NP)
    keys = np.ascontiguousarray(np.asarray(keys, dtype=np.float32))
    vals = np.asarray(values, dtype=np.float32).astype(BF16NP)
    ident = np.eye(128, dtype=np.float32)
    common = dict(w1t=w["w1t"], convw=w["convw"], wqk=w["wqk"], wovt=w["wovt"],
                  w23t=w["w23t"], b23=w["b23"], biases=w["biases"],
                  identr=ident, identf=ident,
                  identb=ident.astype(BF16NP))
    in_maps = []
    for c in range(NCORES):
        sl = slice(c * PB, (c + 1) * PB)
        in_maps.append(dict(xin=xin[sl], keys=keys[sl], vals=vals[sl], **common))
    res = bass_utils.run_bass_kernel_spmd(
        nc, in_maps, core_ids=list(range(NCORES)), trace=trace)
    outs = np.concatenate([r["out"] for r in res.results], axis=0)
    states = np.concatenate([r["states"] for r in res.results], axis=0)
    done = np.concatenate([r["done"] for r in res.results], axis=0)[..., None]
    a1 = np.concatenate([np.asarray(r["align1"], dtype=np.float32)
                         for r in res.results], axis=0)
    a4 = np.concatenate([np.asarray(r["align4"], dtype=np.float32)
                         for r in res.results], axis=0)
    return (outs, states, done, a1, a4), res


def kernel(inputs, keys, values, params, prev_max_attention_idx=None):
    out, _ = run(inputs, keys, values, params)
    return out


# revision 30
# speedup vs baseline: 1.0132x; 1.0132x over previous
"""Trainium2 Bass kernel for nn_Decoder (sparse_attention).

Data-parallel over batch: B=64 split across 8 NeuronCores (8 batch elems each).
Per core, the full decoder runs with a channel-major layout ([C, T] on-chip):

  fc1 -> conv1(GLU,res) -> attn1 -> conv2..4 -> attn4 -> fc2/fc3 heads

Key layout choices:
  - x is kept channel-major [256, T=512] (2 partition tiles) through the net;
    conv taps become plain matmuls over shifted time slices of a left-padded
    SBUF buffer, attention Q/out projections read/write the same layout.
  - attn folds Wq/Wk into Wqk = Wq^T@Wk (bk drops out of softmax) and
    Wv/Wo into Wov = Wo@Wv (bv folds into the output bias via softmax sum=1),
    so keys/values are used raw: scores = (x^T Wqk) keys^T, out = Wov(attn V).
  - keys^T is produced on-chip via PE transposes (fp32r); attn^T likewise
    (bf16) to feed the attn@V matmul, which uses values in natural layout.
  - softmax skips max-subtraction (scores are in [-3, 3]) and gets row sums
    for free from the ACT Exp accum_out port.
  - dtypes: fp32r (tf32-grade, full PE speed) for the score path,
    bf16 for conv/projection weights and the x chain (SBUF capacity),
    f32 accumulation everywhere (PSUM).

Scheduling (Tile executes each engine's stream strictly in order, so emission
order is the schedule): batch elements are emitted in interleaved pairs so the
partner's matmuls cover every serial ACT/DVE tail (conv GLU, softmax);
attention is emitted in four sub-phases (q2 / scores+softmax / attn-transpose
/ A+proj) alternating between the pair so no PE instruction ever queues
behind a stalled one (head-of-line blocking); the attn@V accumulation is
st-major across all four output psums so PE consumption tracks the DVE
attn^T-evacuation rate; each pair's fc1+keys stages are hoisted before the
previous pair's output stage; weights load on the sync ring split
early/mid/late around the first input loads; outputs/aligns go out via the
otherwise-idle GpSimd SWDGE ring.

Measured on 8 axon-tunneled NeuronCores: ~733 us HW exec (728-738 over runs),
worst output rel-err 6.9e-3 (align1), PE ~96% busy within the span.
"""

import math
import numpy as np
import ml_dtypes

import concourse.bacc as bacc
import concourse.bass as bass
import concourse.tile as tile
import concourse.mybir as mybir
from concourse import bass_utils

F32 = mybir.dt.float32
F32R = mybir.dt.float32r
BF16 = mybir.dt.bfloat16
AF = mybir.ActivationFunctionType
ALU = mybir.AluOpType
BF16NP = ml_dtypes.bfloat16

NCORES = 8
B, T, S = 64, 512, 1024
PB = B // NCORES            # batch elems per core
DD, DE = 256, 512           # decoder dim / encoder dim
INCH = 320                  # fc1 in, fc2 out
KW = 5                      # conv kernel width
DILS = [1, 2, 2, 3]
PADS = [(KW - 1) * d for d in DILS]   # 4, 8, 8, 12
C = math.sqrt(0.5)
SQS = math.sqrt(S)

_cached = {}


# --------------------------------------------------------------------------
# host-side weight packing
# --------------------------------------------------------------------------

def _np(x):
    return np.asarray(x, dtype=np.float32)


def _pack_weights(params):
    p = params
    w = {}
    w["w1t"] = _np(p["fc1_W"]).T.copy().astype(BF16NP)            # [320, 256]

    convw = np.zeros((4, 2, 128, KW, 2 * DD), dtype=np.float32)
    for i in range(4):
        Wc = _np(p["convs"][i][0])                                 # [512, 256, 5]
        Ws = np.concatenate([Wc[:DD] * C, Wc[DD:]], axis=0)        # scale a-half
        convw[i] = Ws.transpose(1, 2, 0).reshape(2, 128, KW, 2 * DD)
    w["convw"] = convw.astype(BF16NP)

    wqk = np.zeros((2, 2, 128, DE), dtype=np.float32)
    wovt = np.zeros((2, 4, 128, DD), dtype=np.float32)
    bqk = np.zeros((2, DE), dtype=np.float32)
    bov = np.zeros((2, DD), dtype=np.float32)
    for j, key in enumerate(("attn1", "attn4")):
        ap = p[key]
        Wq, bq = _np(ap["Wq"]), _np(ap["bq"])
        Wk = _np(ap["Wk"])
        Wv, bv = _np(ap["Wv"]), _np(ap["bv"])
        Wo, bo = _np(ap["Wo"]), _np(ap["bo"])
        wqk[j] = (Wq.T @ Wk).reshape(2, 128, DE)
        bqk[j] = bq @ Wk
        wov = (C * SQS) * (Wo @ Wv)                                # [256, 512]
        wovt[j] = wov.T.reshape(4, 128, DD)
        bov[j] = C * (SQS * (Wo @ bv) + bo)
    w["wqk"] = wqk.astype(BF16NP)
    w["wovt"] = wovt.astype(BF16NP)

    w23 = np.concatenate([_np(p["fc2_W"]), _np(p["fc3_W"])], axis=0)  # [321, 256]
    w["w23t"] = w23.T.reshape(2, 128, 321).astype(BF16NP)
    w["b23"] = np.concatenate([_np(p["fc2_b"]), _np(p["fc3_b"])])[None, :] \
        .astype(BF16NP)                                            # [1, 321]

    # per-partition biases, packed [128, n] column-per-bias
    cols = []
    b1 = _np(p["fc1_b"])
    cols += [b1[:128], b1[128:]]                                   # 0..1
    for i in range(4):
        bc = _np(p["convs"][i][1])
        cols += [bc[:128] * C, bc[128:256] * C]                    # a-half (scaled)
        cols += [bc[256:384], bc[384:]]                            # g-half
    for j in range(2):                                             # 18..25
        for d4 in range(4):
            cols.append(bqk[j, d4 * 128:(d4 + 1) * 128])
    for j in range(2):                                             # 26..29
        for ct in range(2):
            cols.append(bov[j, ct * 128:(ct + 1) * 128])
    w["biases"] = np.stack(cols, axis=1).astype(np.float32)        # [128, 30]
    return w

BIAS_B1 = 0
def _bias_conv_a(i, h): return 2 + i * 4 + h
def _bias_conv_g(i, h): return 2 + i * 4 + 2 + h
def _bias_qk(j, d4): return 18 + j * 4 + d4
def _bias_ov(j, ct): return 26 + j * 2 + ct


# --------------------------------------------------------------------------
# device program
# --------------------------------------------------------------------------

def _build():
    nc = bacc.Bacc("TRN2", target_bir_lowering=False, debug=False,
                   enable_asserts=False, num_devices=NCORES)

    d_xin = nc.dram_tensor("xin", [PB, T, INCH], BF16, kind="ExternalInput")
    d_keys = nc.dram_tensor("keys", [PB, S, DE], F32R, kind="ExternalInput")
    d_vals = nc.dram_tensor("vals", [PB, S, DE], BF16, kind="ExternalInput")
    d_w1t = nc.dram_tensor("w1t", [INCH, DD], BF16, kind="ExternalInput")
    d_convw = nc.dram_tensor("convw", [4, 2, 128, KW, 2 * DD], BF16, kind="ExternalInput")
    d_wqk = nc.dram_tensor("wqk", [2, 2, 128, DE], BF16, kind="ExternalInput")
    d_wovt = nc.dram_tensor("wovt", [2, 4, 128, DD], BF16, kind="ExternalInput")
    d_w23t = nc.dram_tensor("w23t", [2, 128, 321], BF16, kind="ExternalInput")
    d_b23 = nc.dram_tensor("b23", [1, 321], BF16, kind="ExternalInput")
    d_biases = nc.dram_tensor("biases", [128, 30], F32, kind="ExternalInput")
    d_identr = nc.dram_tensor("identr", [128, 128], F32R, kind="ExternalInput")
    d_identf = nc.dram_tensor("identf", [128, 128], F32, kind="ExternalInput")
    d_identb = nc.dram_tensor("identb", [128, 128], BF16, kind="ExternalInput")

    d_out = nc.dram_tensor("out", [PB, T, INCH], F32, kind="ExternalOutput")
    d_states = nc.dram_tensor("states", [PB, T, DD], F32, kind="ExternalOutput")
    d_done = nc.dram_tensor("done", [PB, T], F32, kind="ExternalOutput")
    d_align = [nc.dram_tensor("align1", [PB, T, S], BF16, kind="ExternalOutput"),
               nc.dram_tensor("align4", [PB, T, S], BF16, kind="ExternalOutput")]

    with tile.TileContext(nc) as tc:
        _emit(nc, tc, d_xin, d_keys, d_vals, d_w1t, d_convw, d_wqk, d_wovt,
              d_w23t, d_b23, d_biases, d_identr, d_identf, d_identb,
              d_out, d_states, d_done, d_align)
    nc.compile()
    return nc


def _emit(nc, tc, d_xin, d_keys, d_vals, d_w1t, d_convw, d_wqk, d_wovt,
          d_w23t, d_b23, d_biases, d_identr, d_identf, d_identb,
          d_out, d_states, d_done, d_align):
    from contextlib import ExitStack
    ctx = ExitStack()
    wp = ctx.enter_context(tc.tile_pool(name="weights", bufs=1))
    xp = ctx.enter_context(tc.tile_pool(name="xchain", bufs=2))
    kp = ctx.enter_context(tc.tile_pool(name="kv", bufs=2))
    ap_ = ctx.enter_context(tc.tile_pool(name="attn", bufs=2))
    sp = ctx.enter_context(tc.tile_pool(name="small", bufs=2))
    op = ctx.enter_context(tc.tile_pool(name="outs", bufs=2))
    ps = ctx.enter_context(tc.tile_pool(name="ps", bufs=6, space="PSUM"))
    pt = ctx.enter_context(tc.tile_pool(name="pstr", bufs=2, space="PSUM"))

    # ---- pair-0 inputs first on the sync ring (first matmul gates on them),
    # then the critical weights; the bulk (convw[1:], attn4/fc2 weights) is
    # emitted after pair 0's keys loads, arriving well before first use -----
    pre_inA = {}
    for b in (0, 1):
        t_ = sp.tile([128, 4 * INCH], BF16, tag="inA", name=f"inA{b}")
        nc.sync.dma_start(
            t_[:], d_xin.ap()[b].rearrange("(tt tp) c -> tp tt c", tp=128))
        pre_inA[b] = t_
    identb = wp.tile([128, 128], BF16, tag="identb", name="identb")
    nc.sync.dma_start(identb[:], d_identb.ap())
    w1t_sb = []
    for kk in range(3):
        kw_ = 64 if kk == 2 else 128
        t_ = wp.tile([kw_, DD], BF16, tag=f"w1t{kk}", name=f"w1t{kk}")
        nc.sync.dma_start(t_[:], d_w1t.ap()[kk * 128:kk * 128 + kw_, :])
        w1t_sb.append(t_)
    bias_sb = wp.tile([128, 30], F32, tag="bias", name="bias")
    nc.sync.dma_start(bias_sb[:], d_biases.ap())
    identr = wp.tile([128, 128], F32R, tag="identr", name="identr")
    convw_sb = [[wp.tile([128, KW * 2 * DD], BF16, tag=f"cw{i}{ict}",
                         name=f"cw{i}{ict}")
                 for ict in range(2)] for i in range(4)]
    wqk_sb = [[wp.tile([128, DE], BF16, tag=f"wqk{j}{ct}", name=f"wqk{j}{ct}")
               for ct in range(2)] for j in range(2)]
    wovt_sb = [[wp.tile([128, DD], BF16, tag=f"wov{j}{d4}", name=f"wov{j}{d4}")
                for d4 in range(4)] for j in range(2)]
    w23t_sb = [wp.tile([128, 321], BF16, tag=f"w23{ct}", name=f"w23{ct}")
               for ct in range(2)]
    b23_sb = wp.tile([1, 321], BF16, tag="b23", name="b23")
    identf = wp.tile([128, 128], F32, tag="identf", name="identf")
    onesb = wp.tile([1, 128], BF16, tag="ones", name="ones")
    nc.vector.memset(onesb[:], 1.0)

    def load_mid_weights():
        nc.sync.dma_start(identr[:], d_identr.ap())
        for ict in range(2):
            nc.sync.dma_start(convw_sb[0][ict][:], d_convw.ap()[0, ict])
        for ct in range(2):
            nc.sync.dma_start(wqk_sb[0][ct][:], d_wqk.ap()[0, ct])
        for d4 in range(4):
            nc.sync.dma_start(wovt_sb[0][d4][:], d_wovt.ap()[0, d4])

    def load_late_weights():
        for i in range(1, 4):
            for ict in range(2):
                nc.sync.dma_start(convw_sb[i][ict][:], d_convw.ap()[i, ict])
        for ct in range(2):
            nc.sync.dma_start(wqk_sb[1][ct][:], d_wqk.ap()[1, ct])
        for d4 in range(4):
            nc.sync.dma_start(wovt_sb[1][d4][:], d_wovt.ap()[1, d4])
        for ct in range(2):
            nc.sync.dma_start(w23t_sb[ct][:], d_w23t.ap()[ct])
        nc.sync.dma_start(b23_sb[:], d_b23.ap())
        nc.sync.dma_start(identf[:], d_identf.ap())

    def bias(i):
        return bias_sb[:, i:i + 1]

    # ---- stages -------------------------------------------------------------
    def load_inA(b):
        inA = sp.tile([128, 4 * INCH], BF16, tag="inA", name=f"inA{b}")
        nc.sync.dma_start(
            inA[:], d_xin.ap()[b].rearrange("(tt tp) c -> tp tt c", tp=128))
        return inA

    def fc1_stage(b, xp1, inA):
        inT = []
        for cc in range(3):
            cw = 64 if cc == 2 else 128
            pmt = pt.tile([128, T], BF16, tag="tr", name=f"inT{b}{cc}")
            for tt in range(4):
                nc.tensor.matmul(
                    pmt[:cw, tt * 128:tt * 128 + 128],
                    inA[:, tt * INCH + cc * 128:tt * INCH + cc * 128 + cw],
                    identb[:], is_transpose=True)
            it = sp.tile([128, T], BF16, tag=f"inT{cc}", name=f"inTs{b}{cc}")
            nc.vector.tensor_copy(it[:cw, :], pmt[:cw, :])
            inT.append(it)
        for ct in range(2):
            pm = ps.tile([128, T], F32, tag="mm", name=f"fc1{b}{ct}")
            for kk in range(3):
                kw_ = 64 if kk == 2 else 128
                nc.tensor.matmul(pm[:],
                                 w1t_sb[kk][:kw_, ct * 128:ct * 128 + 128],
                                 inT[kk][:kw_, :],
                                 start=(kk == 0), stop=(kk == 2))
            nc.vector.memset(xp1[ct][:, 0:PADS[0]], 0.0)
            nc.scalar.activation(xp1[ct][:, PADS[0]:PADS[0] + T], pm[:], AF.Relu,
                                 bias=bias(BIAS_B1 + ct))
        return xp1

    def kv_stage(b):
        keysT = [kp.tile([128, S], F32R, tag=f"keysT{d4}", name=f"kT{b}{d4}")
                 for d4 in range(4)]
        krr = d_keys.ap()[b].rearrange("(st sp) d -> sp st d", sp=128)
        for half in range(2):
            kn2 = []
            for q in range(2):
                knat = kp.tile([128, 2 * DE], F32R, tag="knat",
                               name=f"kn{b}{half}{q}", bufs=2)
                nc.sync.dma_start(
                    knat[:], krr[:, half * 4 + q * 2:half * 4 + q * 2 + 2, :])
                kn2.append(knat)
            for d4 in range(4):
                pmt = pt.tile([128, 512], F32R, tag="tr",
                              name=f"kt{b}{half}{d4}")
                for l in range(4):
                    nc.tensor.matmul(pmt[:, l * 128:l * 128 + 128],
                                     kn2[l // 2][:, (l % 2) * DE + d4 * 128:
                                                 (l % 2) * DE + d4 * 128 + 128],
                                     identr[:], is_transpose=True)
                nc.vector.tensor_copy(keysT[d4][:, half * 512:half * 512 + 512],
                                      pmt[:])
        vnat = kp.tile([128, 8 * DE], BF16, tag="vnat", name=f"vn{b}")
        nc.sync.dma_start(
            vnat[:], d_vals.ap()[b].rearrange("(st sp) d -> sp st d", sp=128))
        return keysT, vnat

    def conv_block(i, b, src, soff, dst, doff):
        d = DILS[i]
        pms = {}
        for oct in (2, 3, 0, 1):          # g-half first: sigmoids overlap a-half
            pm = ps.tile([128, T], F32, tag="mm", name=f"cv{b}_{i}o{oct}")
            n = 0
            for ict in range(2):
                for k in range(KW):
                    nc.tensor.matmul(
                        pm[:],
                        convw_sb[i][ict][:, k * 2 * DD + oct * 128:
                                         k * 2 * DD + oct * 128 + 128],
                        src[ict][:, k * d:k * d + T],
                        start=(n == 0), stop=(n == 9))
                    n += 1
            pms[oct] = pm
        for h in range(2):
            sig = sp.tile([128, T], BF16, tag=f"sig{h}", name=f"cv{b}_{i}s{h}")
            nc.scalar.activation(sig[:], pms[2 + h][:], AF.Sigmoid,
                                 bias=bias(_bias_conv_g(i, h)))
            glu = sp.tile([128, T], BF16, tag=f"glu{h}", name=f"cv{b}_{i}g{h}")
            nc.vector.scalar_tensor_tensor(
                glu[:], pms[h][:], bias(_bias_conv_a(i, h)), sig[:],
                ALU.add, ALU.mult)
            nc.vector.scalar_tensor_tensor(
                dst[h][:, doff:doff + T], src[h][:, soff:soff + T], C, glu[:],
                ALU.mult, ALU.add)

    def attn_q2(j, b, src, soff):
        q2T = []
        for d4 in range(4):
            pm = ps.tile([128, T], F32, tag="mm", name=f"a{b}_{j}q{d4}")
            for ct in range(2):
                nc.tensor.matmul(pm[:],
                                 wqk_sb[j][ct][:, d4 * 128:d4 * 128 + 128],
                                 src[ct][:, soff:soff + T],
                                 start=(ct == 0), stop=(ct == 1))
            qt = ap_.tile([128, T], F32R, tag=f"q2T{d4}", name=f"a{b}_{j}qt{d4}")
            nc.scalar.activation(qt[:], pm[:], AF.Identity,
                                 bias=bias(_bias_qk(j, d4)))
            q2T.append(qt)
        return q2T

    def attn_scores(j, b, q2T, keysT):
        attnb = []
        for tt in range(4):
            eb = ap_.tile([128, S], BF16, tag=f"exp{tt}", name=f"a{b}_{j}e{tt}")
            zp = sp.tile([128, 2], F32, tag="zp", name=f"a{b}_{j}zp{tt}")
            for sh in range(2):
                pm = ps.tile([128, 512], F32, tag="mm", name=f"a{b}_{j}s{tt}{sh}")
                for d4 in range(4):
                    nc.tensor.matmul(pm[:],
                                     q2T[d4][:, tt * 128:tt * 128 + 128],
                                     keysT[d4][:, sh * 512:sh * 512 + 512],
                                     start=(d4 == 0), stop=(d4 == 3))
                nc.scalar.activation(eb[:, sh * 512:sh * 512 + 512], pm[:],
                                     AF.Exp, accum_out=zp[:, sh:sh + 1])
            z = sp.tile([128, 1], F32, tag="z", name=f"a{b}_{j}z{tt}")
            nc.vector.tensor_add(z[:], zp[:, 0:1], zp[:, 1:2])
            rec = sp.tile([128, 1], F32, tag="rec", name=f"a{b}_{j}r{tt}")
            nc.vector.reciprocal(rec[:], z[:])
            nc.vector.tensor_scalar_mul(eb[:], eb[:], rec[:])
            nc.gpsimd.dma_start(d_align[j].ap()[b, tt * 128:tt * 128 + 128, :],
                                eb[:])
            attnb.append(eb)
        return attnb

    def attn_tr(j, b, attnb):
        attnT = []
        for sg in range(4):
            pmt = pt.tile([128, 2 * T], BF16, tag="tr", name=f"a{b}_{j}t{sg}")
            for l in range(2):
                st = 2 * sg + l
                for tt in range(4):
                    nc.tensor.matmul(
                        pmt[:, l * T + tt * 128:l * T + tt * 128 + 128],
                        attnb[tt][:, st * 128:st * 128 + 128],
                        identb[:], is_transpose=True)
            at = ap_.tile([128, 2 * T], BF16, tag=f"attnT{sg}",
                          name=f"a{b}_{j}at{sg}")
            nc.vector.tensor_copy(at[:], pmt[:])
            attnT.append(at)
        return attnT

    def attn_av(j, b, attnT, vnat, src, soff, dst, doff):
        pmA = [ps.tile([128, T], F32, tag="mm", name=f"a{b}_{j}A{d4}")
               for d4 in range(4)]
        for st in range(8):
            for d4 in range(4):
                nc.tensor.matmul(pmA[d4][:],
                                 vnat[:, st * DE + d4 * 128:
                                      st * DE + d4 * 128 + 128],
                                 attnT[st // 2][:, (st % 2) * T:
                                                (st % 2) * T + T],
                                 start=(st == 0), stop=(st == 7))
        Asb = []
        for d4 in range(4):
            asb = ap_.tile([128, T], BF16, tag=f"Asb{d4}", name=f"a{b}_{j}as{d4}")
            nc.scalar.copy(asb[:], pmA[d4][:])
            Asb.append(asb)
        for ct in range(2):
            pm = ps.tile([128, T], F32, tag="mm", name=f"a{b}_{j}p{ct}")
            for d4 in range(4):
                nc.tensor.matmul(pm[:],
                                 wovt_sb[j][d4][:, ct * 128:ct * 128 + 128],
                                 Asb[d4][:],
                                 start=(d4 == 0), stop=(d4 == 3))
            tmp = sp.tile([128, T], BF16, tag=f"ptmp{ct}", name=f"a{b}_{j}pt{ct}")
            nc.scalar.activation(tmp[:], pm[:], AF.Identity,
                                 bias=bias(_bias_ov(j, ct)))
            nc.vector.scalar_tensor_tensor(
                dst[ct][:, doff:doff + T], src[ct][:, soff:soff + T], C,
                tmp[:], ALU.mult, ALU.add)

    def out_stage(b, scm):
        for tt in range(4):
            pmt = pt.tile([128, DD], BF16, tag="tr", name=f"st{b}{tt}")
            for ct in range(2):
                nc.tensor.matmul(pmt[:, ct * 128:ct * 128 + 128],
                                 scm[ct][:, tt * 128:tt * 128 + 128],
                                 identb[:], is_transpose=True)
            stg = op.tile([128, DD], F32, tag="st_sb", name=f"stsb{b}{tt}")
            nc.vector.tensor_copy(stg[:], pmt[:])
            nc.gpsimd.dma_start(
                d_states.ap()[b, tt * 128:tt * 128 + 128, :], stg[:])
        dcol = op.tile([128, 4], F32, tag="dcol", name=f"dcol{b}")
        for tt in range(4):
            pm = ps.tile([128, 321], F32, tag="mm", name=f"fc2{b}{tt}")
            for ct in range(2):
                nc.tensor.matmul(pm[:],
                                 scm[ct][:, tt * 128:tt * 128 + 128],
                                 w23t_sb[ct][:],
                                 start=(ct == 0), stop=False)
            nc.tensor.matmul(pm[:], onesb[:, 0:128],
                             b23_sb[:], start=False, stop=True)
            nc.vector.tensor_copy(dcol[:, tt:tt + 1], pm[:, 320:321])
            osb = op.tile([128, 321], F32, tag="osb", name=f"osb{b}{tt}")
            nc.scalar.activation(osb[:], pm[:], AF.Sigmoid)
            nc.gpsimd.dma_start(
                d_out.ap()[b, tt * 128:tt * 128 + 128, :], osb[:, 0:INCH])
        pmt = pt.tile([128, 128], F32, tag="tr", name=f"dn{b}")
        nc.tensor.matmul(pmt[:4, :], dcol[:], identf[:], is_transpose=True)
        done_sb = op.tile([4, 128], F32, tag="done", name=f"dnsb{b}")
        nc.scalar.activation(done_sb[:], pmt[:4, :], AF.Sigmoid)
        nc.gpsimd.dma_start(
            d_done.ap()[b].rearrange("(n p) -> n p", n=4), done_sb[:])

    # ---- interleaved batch-element pairs: the partner's matmuls fill each
    # stage's serial ACT/DVE tail so the PE never drains ----------------------
    def xtile(side, b, k):
        return [xp.tile([128, 524], BF16, tag=f"xs{side}{ct}",
                        name=f"xs{side}_{b}{k}{ct}") for ct in range(2)]

    pend_out = []          # (b, scm) of the previous pair, not yet emitted
    for pair in range(PB // 2):
        bs = (2 * pair, 2 * pair + 1)
        st = {}
        ina = {b: (pre_inA[b] if pair == 0 else load_inA(b)) for b in bs}
        for b in bs:
            xp1 = xtile("A", b, 1)
            st[b] = {"xp1": fc1_stage(b, xp1, ina[b])}
        if pair == 0:
            load_mid_weights()
        for b in bs:
            st[b]["kv"] = kv_stage(b)
        if pair == 0:
            load_late_weights()
        for b, scm in pend_out:
            out_stage(b, scm)
        pend_out = []
        for b in bs:
            x = st[b]
            x["xc1"] = xtile("B", b, 1)
            conv_block(0, b, x["xp1"], PADS[0], x["xc1"], 0)
        for b in bs:
            x = st[b]
            x["q1"] = attn_q2(0, b, x["xc1"], 0)
        for b in bs:
            x = st[b]
            x["ab1"] = attn_scores(0, b, x["q1"], x["kv"][0])
        for b in bs:
            x = st[b]
            x["at1"] = attn_tr(0, b, x["ab1"])
        for b in bs:
            x = st[b]
            x["xp2"] = xtile("A", b, 2)
            for ct in range(2):
                nc.vector.memset(x["xp2"][ct][:, 0:PADS[1]], 0.0)
            keysT, vnat = x["kv"]
            attn_av(0, b, x["at1"], vnat, x["xc1"], 0, x["xp2"], PADS[1])
        for b in bs:
            x = st[b]
            x["xp3"] = xtile("B", b, 2)
            for ct in range(2):
                nc.vector.memset(x["xp3"][ct][:, 0:PADS[2]], 0.0)
            conv_block(1, b, x["xp2"], PADS[1], x["xp3"], PADS[2])
        for b in bs:
            x = st[b]
            x["xp4"] = xtile("A", b, 3)
            for ct in range(2):
                nc.vector.memset(x["xp4"][ct][:, 0:PADS[3]], 0.0)
            conv_block(2, b, x["xp3"], PADS[2], x["xp4"], PADS[3])
        for b in bs:
            x = st[b]
            x["xc4"] = xtile("B", b, 3)
            conv_block(3, b, x["xp4"], PADS[3], x["xc4"], 0)
        for b in bs:
            x = st[b]
            x["q4"] = attn_q2(1, b, x["xc4"], 0)
        for b in bs:
            x = st[b]
            x["ab4"] = attn_scores(1, b, x["q4"], x["kv"][0])
        for b in bs:
            x = st[b]
            x["at4"] = attn_tr(1, b, x["ab4"])
        for b in bs:
            x = st[b]
            x["scm"] = [xp.tile([128, T], BF16, tag=f"scm{ct}",
                                name=f"scm{b}{ct}") for ct in range(2)]
            keysT, vnat = x["kv"]
            attn_av(1, b, x["at4"], vnat, x["xc4"], 0, x["scm"], 0)
            pend_out.append((b, x["scm"]))
    for b, scm in pend_out:
        out_stage(b, scm)

    ctx.close()


# --------------------------------------------------------------------------
# entry point
# --------------------------------------------------------------------------

def _get_program():
    if "nc" not in _cached:
        _cached["nc"] = _build()
    return _cached["nc"]


def run(inputs, keys, values, params, trace=False):
    nc = _get_program()
    w = _pack_weights(params)
    xin = np.asarray(inputs, dtype=np.float32).astype(BF16NP)
    keys = np.ascontiguousarray(np.asarray(keys, dtype=np.float32))
    vals = np.asarray(values, dtype=np.float32).astype(BF16NP)
    ident = np.eye(128, dtype=np.float32)
    common = dict(w1t=w["w1t"], convw=w["convw"], wqk=w["wqk"], wovt=w["wovt"],
                  w23t=w["w23t"], b23=w["b23"], biases=w["biases"],
                  identr=ident, identf=ident,
                  identb=ident.astype(BF16NP))
    in_maps = []
    for c in range(NCORES):
        sl = slice(c * PB, (c + 1) * PB)
        in_maps.append(dict(xin=xin[sl], keys=keys[sl], vals=vals[sl], **common))
    res = bass_utils.run_bass_kernel_spmd(
        nc, in_maps, core_ids=list(range(NCORES)), trace=trace)
    outs = np.concatenate([r["out"] for r in res.results], axis=0)
    states = np.concatenate([r["states"] for r in res.results], axis=0)
    done = np.concatenate([r["done"] for r in res.results], axis=0)[..., None]
    a1 = np.concatenate([np.asarray(r["align1"], dtype=np.float32)
                         for r in res.results], axis=0)
    a4 = np.concatenate([np.asarray(r["align4"], dtype=np.float32)
                         for r in res.results], axis=0)
    return (outs, states, done, a1, a4), res


def kernel(inputs, keys, values, params, prev_max_attention_idx=None):
    out, _ = run(inputs, keys, values, params)
    return out


# revision 31
# speedup vs baseline: 1.0299x; 1.0164x over previous
"""Trainium2 Bass kernel for nn_Decoder (sparse_attention).

Data-parallel over batch: B=64 split across 8 NeuronCores (8 batch elems each).
Per core, the full decoder runs with a channel-major layout ([C, T] on-chip):

  fc1 -> conv1(GLU,res) -> attn1 -> conv2..4 -> attn4 -> fc2/fc3 heads

Key layout choices:
  - x is kept channel-major [256, T=512] (2 partition tiles) through the net;
    conv taps become plain matmuls over shifted time slices of a left-padded
    SBUF buffer, attention Q/out projections read/write the same layout.
  - attn folds Wq/Wk into Wqk = Wq^T@Wk (bk drops out of softmax) and
    Wv/Wo into Wov = Wo@Wv (bv folds into the output bias via softmax sum=1),
    so keys/values are used raw: scores = (x^T Wqk) keys^T, out = Wov(attn V).
  - keys^T is produced on-chip via PE transposes (fp32r); attn^T likewise
    (bf16) to feed the attn@V matmul, which uses values in natural layout.
  - softmax skips max-subtraction (scores are in [-3, 3]) and gets row sums
    for free from the ACT Exp accum_out port.
  - dtypes: fp32r (tf32-grade, full PE speed) for the score path,
    bf16 for conv/projection weights and the x chain (SBUF capacity),
    f32 accumulation everywhere (PSUM).

Scheduling (Tile executes each engine's stream strictly in order, so emission
order is the schedule): batch elements are emitted in interleaved pairs so the
partner's matmuls cover every serial ACT/DVE tail (conv GLU, softmax);
attention is emitted in four sub-phases (q2 / scores+softmax / attn-transpose
/ A+proj) alternating between the pair so no PE instruction ever queues
behind a stalled one (head-of-line blocking); the attn@V accumulation is
st-major across all four output psums so PE consumption tracks the DVE
attn^T-evacuation rate; each pair's fc1+keys stages are hoisted before the
previous pair's output stage; weights load on the sync ring split
early/mid/late around the first input loads; outputs/aligns go out via the
otherwise-idle GpSimd SWDGE ring.

Measured on 8 axon-tunneled NeuronCores: ~733 us HW exec (728-738 over runs),
worst output rel-err 6.9e-3 (align1), PE ~96% busy within the span.
"""

import math
import numpy as np
import ml_dtypes

import concourse.bacc as bacc
import concourse.bass as bass
import concourse.tile as tile
import concourse.mybir as mybir
from concourse import bass_utils

F32 = mybir.dt.float32
F32R = mybir.dt.float32r
BF16 = mybir.dt.bfloat16
AF = mybir.ActivationFunctionType
ALU = mybir.AluOpType
BF16NP = ml_dtypes.bfloat16

NCORES = 8
B, T, S = 64, 512, 1024
PB = B // NCORES            # batch elems per core
DD, DE = 256, 512           # decoder dim / encoder dim
INCH = 320                  # fc1 in, fc2 out
KW = 5                      # conv kernel width
DILS = [1, 2, 2, 3]
PADS = [(KW - 1) * d for d in DILS]   # 4, 8, 8, 12
C = math.sqrt(0.5)
SQS = math.sqrt(S)

_cached = {}


# --------------------------------------------------------------------------
# host-side weight packing
# --------------------------------------------------------------------------

def _np(x):
    return np.asarray(x, dtype=np.float32)


def _pack_weights(params):
    p = params
    w = {}
    w["w1t"] = _np(p["fc1_W"]).T.copy().astype(BF16NP)            # [320, 256]

    convw = np.zeros((4, 2, 128, KW, 2 * DD), dtype=np.float32)
    for i in range(4):
        Wc = _np(p["convs"][i][0])                                 # [512, 256, 5]
        Ws = np.concatenate([Wc[:DD] * C, Wc[DD:]], axis=0)        # scale a-half
        convw[i] = Ws.transpose(1, 2, 0).reshape(2, 128, KW, 2 * DD)
    w["convw"] = convw.astype(BF16NP)

    wqk = np.zeros((2, 2, 128, DE), dtype=np.float32)
    wovt = np.zeros((2, 4, 128, DD), dtype=np.float32)
    bqk = np.zeros((2, DE), dtype=np.float32)
    bov = np.zeros((2, DD), dtype=np.float32)
    for j, key in enumerate(("attn1", "attn4")):
        ap = p[key]
        Wq, bq = _np(ap["Wq"]), _np(ap["bq"])
        Wk = _np(ap["Wk"])
        Wv, bv = _np(ap["Wv"]), _np(ap["bv"])
        Wo, bo = _np(ap["Wo"]), _np(ap["bo"])
        wqk[j] = (Wq.T @ Wk).reshape(2, 128, DE)
        bqk[j] = bq @ Wk
        wov = (C * SQS) * (Wo @ Wv)                                # [256, 512]
        wovt[j] = wov.T.reshape(4, 128, DD)
        bov[j] = C * (SQS * (Wo @ bv) + bo)
    w["wqk"] = wqk.astype(BF16NP)
    w["wovt"] = wovt.astype(BF16NP)

    w23 = np.concatenate([_np(p["fc2_W"]), _np(p["fc3_W"])], axis=0)  # [321, 256]
    w["w23t"] = w23.T.reshape(2, 128, 321).astype(BF16NP)
    w["b23"] = np.concatenate([_np(p["fc2_b"]), _np(p["fc3_b"])])[None, :] \
        .astype(BF16NP)                                            # [1, 321]

    # per-partition biases, packed [128, n] column-per-bias
    cols = []
    b1 = _np(p["fc1_b"])
    cols += [b1[:128], b1[128:]]                                   # 0..1
    for i in range(4):
        bc = _np(p["convs"][i][1])
        cols += [bc[:128] * C, bc[128:256] * C]                    # a-half (scaled)
        cols += [bc[256:384], bc[384:]]                            # g-half
    for j in range(2):                                             # 18..25
        for d4 in range(4):
            cols.append(bqk[j, d4 * 128:(d4 + 1) * 128])
    for j in range(2):                                             # 26..29
        for ct in range(2):
            cols.append(bov[j, ct * 128:(ct + 1) * 128])
    w["biases"] = np.stack(cols, axis=1).astype(np.float32)        # [128, 30]
    return w

BIAS_B1 = 0
def _bias_conv_a(i, h): return 2 + i * 4 + h
def _bias_conv_g(i, h): return 2 + i * 4 + 2 + h
def _bias_qk(j, d4): return 18 + j * 4 + d4
def _bias_ov(j, ct): return 26 + j * 2 + ct


# --------------------------------------------------------------------------
# device program
# --------------------------------------------------------------------------

def _build():
    nc = bacc.Bacc("TRN2", target_bir_lowering=False, debug=False,
                   enable_asserts=False, num_devices=NCORES)

    d_xin = nc.dram_tensor("xin", [PB, T, INCH], BF16, kind="ExternalInput")
    d_keys = nc.dram_tensor("keys", [PB, S, DE], F32R, kind="ExternalInput")
    d_vals = nc.dram_tensor("vals", [PB, S, DE], BF16, kind="ExternalInput")
    d_w1t = nc.dram_tensor("w1t", [INCH, DD], BF16, kind="ExternalInput")
    d_convw = nc.dram_tensor("convw", [4, 2, 128, KW, 2 * DD], BF16, kind="ExternalInput")
    d_wqk = nc.dram_tensor("wqk", [2, 2, 128, DE], BF16, kind="ExternalInput")
    d_wovt = nc.dram_tensor("wovt", [2, 4, 128, DD], BF16, kind="ExternalInput")
    d_w23t = nc.dram_tensor("w23t", [2, 128, 321], BF16, kind="ExternalInput")
    d_b23 = nc.dram_tensor("b23", [1, 321], BF16, kind="ExternalInput")
    d_biases = nc.dram_tensor("biases", [128, 30], F32, kind="ExternalInput")
    d_identr = nc.dram_tensor("identr", [128, 128], F32R, kind="ExternalInput")
    d_identf = nc.dram_tensor("identf", [128, 128], F32, kind="ExternalInput")
    d_identb = nc.dram_tensor("identb", [128, 128], BF16, kind="ExternalInput")

    d_out = nc.dram_tensor("out", [PB, T, INCH], F32, kind="ExternalOutput")
    d_states = nc.dram_tensor("states", [PB, T, DD], F32, kind="ExternalOutput")
    d_done = nc.dram_tensor("done", [PB, T], F32, kind="ExternalOutput")
    d_align = [nc.dram_tensor("align1", [PB, T, S], BF16, kind="ExternalOutput"),
               nc.dram_tensor("align4", [PB, T, S], BF16, kind="ExternalOutput")]

    with tile.TileContext(nc) as tc:
        _emit(nc, tc, d_xin, d_keys, d_vals, d_w1t, d_convw, d_wqk, d_wovt,
              d_w23t, d_b23, d_biases, d_identr, d_identf, d_identb,
              d_out, d_states, d_done, d_align)
    nc.compile()
    return nc


def _emit(nc, tc, d_xin, d_keys, d_vals, d_w1t, d_convw, d_wqk, d_wovt,
          d_w23t, d_b23, d_biases, d_identr, d_identf, d_identb,
          d_out, d_states, d_done, d_align):
    from contextlib import ExitStack
    ctx = ExitStack()
    wp = ctx.enter_context(tc.tile_pool(name="weights", bufs=1))
    xp = ctx.enter_context(tc.tile_pool(name="xchain", bufs=2))
    kp = ctx.enter_context(tc.tile_pool(name="kv", bufs=2))
    ap_ = ctx.enter_context(tc.tile_pool(name="attn", bufs=2))
    sp = ctx.enter_context(tc.tile_pool(name="small", bufs=2))
    op = ctx.enter_context(tc.tile_pool(name="outs", bufs=2))
    ps = ctx.enter_context(tc.tile_pool(name="ps", bufs=6, space="PSUM"))
    pt = ctx.enter_context(tc.tile_pool(name="pstr", bufs=2, space="PSUM"))

    # ---- persistent weights: critical few load first on the sync ring; the
    # bulk (convw[1:], attn4/fc2 weights) is emitted after pair 0's input
    # loads so it queues behind them, arriving well before first use --------
    identb = wp.tile([128, 128], BF16, tag="identb", name="identb")
    nc.sync.dma_start(identb[:], d_identb.ap())
    w1t_sb = []
    for kk in range(3):
        kw_ = 64 if kk == 2 else 128
        t_ = wp.tile([kw_, DD], BF16, tag=f"w1t{kk}", name=f"w1t{kk}")
        nc.sync.dma_start(t_[:], d_w1t.ap()[kk * 128:kk * 128 + kw_, :])
        w1t_sb.append(t_)
    bias_sb = wp.tile([128, 30], F32, tag="bias", name="bias")
    nc.sync.dma_start(bias_sb[:], d_biases.ap())
    identr = wp.tile([128, 128], F32R, tag="identr", name="identr")
    convw_sb = [[wp.tile([128, KW * 2 * DD], BF16, tag=f"cw{i}{ict}",
                         name=f"cw{i}{ict}")
                 for ict in range(2)] for i in range(4)]
    wqk_sb = [[wp.tile([128, DE], BF16, tag=f"wqk{j}{ct}", name=f"wqk{j}{ct}")
               for ct in range(2)] for j in range(2)]
    wovt_sb = [[wp.tile([128, DD], BF16, tag=f"wov{j}{d4}", name=f"wov{j}{d4}")
                for d4 in range(4)] for j in range(2)]
    w23t_sb = [wp.tile([128, 321], BF16, tag=f"w23{ct}", name=f"w23{ct}")
               for ct in range(2)]
    b23_sb = wp.tile([1, 321], BF16, tag="b23", name="b23")
    identf = wp.tile([128, 128], F32, tag="identf", name="identf")
    onesb = wp.tile([1, 128], BF16, tag="ones", name="ones")
    nc.vector.memset(onesb[:], 1.0)

    def load_mid_weights():
        nc.sync.dma_start(identr[:], d_identr.ap())
        for ict in range(2):
            nc.sync.dma_start(convw_sb[0][ict][:], d_convw.ap()[0, ict])
        for ct in range(2):
            nc.sync.dma_start(wqk_sb[0][ct][:], d_wqk.ap()[0, ct])
        for d4 in range(4):
            nc.sync.dma_start(wovt_sb[0][d4][:], d_wovt.ap()[0, d4])

    def load_late_weights():
        for i in range(1, 4):
            for ict in range(2):
                nc.sync.dma_start(convw_sb[i][ict][:], d_convw.ap()[i, ict])
        for ct in range(2):
            nc.sync.dma_start(wqk_sb[1][ct][:], d_wqk.ap()[1, ct])
        for d4 in range(4):
            nc.sync.dma_start(wovt_sb[1][d4][:], d_wovt.ap()[1, d4])
        for ct in range(2):
            nc.sync.dma_start(w23t_sb[ct][:], d_w23t.ap()[ct])
        nc.sync.dma_start(b23_sb[:], d_b23.ap())
        nc.sync.dma_start(identf[:], d_identf.ap())

    def bias(i):
        return bias_sb[:, i:i + 1]

    # ---- stages -------------------------------------------------------------
    def load_inA(b):
        inA = sp.tile([128, 4 * INCH], BF16, tag="inA", name=f"inA{b}")
        nc.sync.dma_start(
            inA[:], d_xin.ap()[b].rearrange("(tt tp) c -> tp tt c", tp=128))
        return inA

    def fc1_stage(b, xp1, inA):
        inT = []
        for cc in range(3):
            cw = 64 if cc == 2 else 128
            pmt = pt.tile([128, T], BF16, tag="tr", name=f"inT{b}{cc}")
            for tt in range(4):
                nc.tensor.matmul(
                    pmt[:cw, tt * 128:tt * 128 + 128],
                    inA[:, tt * INCH + cc * 128:tt * INCH + cc * 128 + cw],
                    identb[:], is_transpose=True)
            it = sp.tile([128, T], BF16, tag=f"inT{cc}", name=f"inTs{b}{cc}")
            nc.vector.tensor_copy(it[:cw, :], pmt[:cw, :])
            inT.append(it)
        for ct in range(2):
            pm = ps.tile([128, T], F32, tag="mm", name=f"fc1{b}{ct}")
            for kk in range(3):
                kw_ = 64 if kk == 2 else 128
                nc.tensor.matmul(pm[:],
                                 w1t_sb[kk][:kw_, ct * 128:ct * 128 + 128],
                                 inT[kk][:kw_, :],
                                 start=(kk == 0), stop=(kk == 2))
            nc.vector.memset(xp1[ct][:, 0:PADS[0]], 0.0)
            nc.scalar.activation(xp1[ct][:, PADS[0]:PADS[0] + T], pm[:], AF.Relu,
                                 bias=bias(BIAS_B1 + ct))
        return xp1

    def kv_stage(b):
        keysT = [kp.tile([128, S], F32R, tag=f"keysT{d4}", name=f"kT{b}{d4}")
                 for d4 in range(4)]
        krr = d_keys.ap()[b].rearrange("(st sp) d -> sp st d", sp=128)
        for half in range(2):
            kn2 = []
            for q in range(2):
                knat = kp.tile([128, 2 * DE], F32R, tag="knat",
                               name=f"kn{b}{half}{q}", bufs=2)
                nc.sync.dma_start(
                    knat[:], krr[:, half * 4 + q * 2:half * 4 + q * 2 + 2, :])
                kn2.append(knat)
            for d4 in range(4):
                pmt = pt.tile([128, 512], F32R, tag="tr",
                              name=f"kt{b}{half}{d4}")
                for l in range(4):
                    nc.tensor.matmul(pmt[:, l * 128:l * 128 + 128],
                                     kn2[l // 2][:, (l % 2) * DE + d4 * 128:
                                                 (l % 2) * DE + d4 * 128 + 128],
                                     identr[:], is_transpose=True)
                nc.vector.tensor_copy(keysT[d4][:, half * 512:half * 512 + 512],
                                      pmt[:])
        vnat = kp.tile([128, 8 * DE], BF16, tag="vnat", name=f"vn{b}")
        nc.sync.dma_start(
            vnat[:], d_vals.ap()[b].rearrange("(st sp) d -> sp st d", sp=128))
        return keysT, vnat

    def conv_block(i, b, src, soff, dst, doff):
        d = DILS[i]
        pms = {}
        for oct in (2, 3, 0, 1):          # g-half first: sigmoids overlap a-half
            pm = ps.tile([128, T], F32, tag="mm", name=f"cv{b}_{i}o{oct}")
            n = 0
            for ict in range(2):
                for k in range(KW):
                    nc.tensor.matmul(
                        pm[:],
                        convw_sb[i][ict][:, k * 2 * DD + oct * 128:
                                         k * 2 * DD + oct * 128 + 128],
                        src[ict][:, k * d:k * d + T],
                        start=(n == 0), stop=(n == 9))
                    n += 1
            pms[oct] = pm
        for h in range(2):
            sig = sp.tile([128, T], BF16, tag=f"sig{h}", name=f"cv{b}_{i}s{h}")
            nc.scalar.activation(sig[:], pms[2 + h][:], AF.Sigmoid,
                                 bias=bias(_bias_conv_g(i, h)))
            glu = sp.tile([128, T], BF16, tag=f"glu{h}", name=f"cv{b}_{i}g{h}")
            nc.vector.scalar_tensor_tensor(
                glu[:], pms[h][:], bias(_bias_conv_a(i, h)), sig[:],
                ALU.add, ALU.mult)
            nc.vector.scalar_tensor_tensor(
                dst[h][:, doff:doff + T], src[h][:, soff:soff + T], C, glu[:],
                ALU.mult, ALU.add)

    def attn_q2(j, b, src, soff):
        q2T = []
        for d4 in range(4):
            pm = ps.tile([128, T], F32, tag="mm", name=f"a{b}_{j}q{d4}")
            for ct in range(2):
                nc.tensor.matmul(pm[:],
                                 wqk_sb[j][ct][:, d4 * 128:d4 * 128 + 128],
                                 src[ct][:, soff:soff + T],
                                 start=(ct == 0), stop=(ct == 1))
            qt = ap_.tile([128, T], F32R, tag=f"q2T{d4}", name=f"a{b}_{j}qt{d4}")
            nc.scalar.activation(qt[:], pm[:], AF.Identity,
                                 bias=bias(_bias_qk(j, d4)))
            q2T.append(qt)
        return q2T

    def attn_scores(j, b, q2T, keysT):
        attnb = []
        for tt in range(4):
            eb = ap_.tile([128, S], BF16, tag=f"exp{tt}", name=f"a{b}_{j}e{tt}")
            zp = sp.tile([128, 2], F32, tag="zp", name=f"a{b}_{j}zp{tt}")
            for sh in range(2):
                pm = ps.tile([128, 512], F32, tag="mm", name=f"a{b}_{j}s{tt}{sh}")
                for d4 in range(4):
                    nc.tensor.matmul(pm[:],
                                     q2T[d4][:, tt * 128:tt * 128 + 128],
                                     keysT[d4][:, sh * 512:sh * 512 + 512],
                                     start=(d4 == 0), stop=(d4 == 3))
                nc.scalar.activation(eb[:, sh * 512:sh * 512 + 512], pm[:],
                                     AF.Exp, accum_out=zp[:, sh:sh + 1])
            z = sp.tile([128, 1], F32, tag="z", name=f"a{b}_{j}z{tt}")
            nc.vector.tensor_add(z[:], zp[:, 0:1], zp[:, 1:2])
            rec = sp.tile([128, 1], F32, tag="rec", name=f"a{b}_{j}r{tt}")
            nc.vector.reciprocal(rec[:], z[:])
            nc.vector.tensor_scalar_mul(eb[:], eb[:], rec[:])
            nc.gpsimd.dma_start(d_align[j].ap()[b, tt * 128:tt * 128 + 128, :],
                                eb[:])
            attnb.append(eb)
        return attnb

    def attn_tr(j, b, attnb):
        attnT = []
        for sg in range(4):
            pmt = pt.tile([128, 2 * T], BF16, tag="tr", name=f"a{b}_{j}t{sg}")
            for l in range(2):
                st = 2 * sg + l
                for tt in range(4):
                    nc.tensor.matmul(
                        pmt[:, l * T + tt * 128:l * T + tt * 128 + 128],
                        attnb[tt][:, st * 128:st * 128 + 128],
                        identb[:], is_transpose=True)
            at = ap_.tile([128, 2 * T], BF16, tag=f"attnT{sg}",
                          name=f"a{b}_{j}at{sg}")
            nc.vector.tensor_copy(at[:], pmt[:])
            attnT.append(at)
        return attnT

    def attn_av(j, b, attnT, vnat, src, soff, dst, doff):
        pmA = [ps.tile([128, T], F32, tag="mm", name=f"a{b}_{j}A{d4}")
               for d4 in range(4)]
        for st in range(8):
            for d4 in range(4):
                nc.tensor.matmul(pmA[d4][:],
                                 vnat[:, st * DE + d4 * 128:
                                      st * DE + d4 * 128 + 128],
                                 attnT[st // 2][:, (st % 2) * T:
                                                (st % 2) * T + T],
                                 start=(st == 0), stop=(st == 7))
        Asb = []
        for d4 in range(4):
            asb = ap_.tile([128, T], BF16, tag=f"Asb{d4}", name=f"a{b}_{j}as{d4}")
            nc.scalar.copy(asb[:], pmA[d4][:])
            Asb.append(asb)
        for ct in range(2):
            pm = ps.tile([128, T], F32, tag="mm", name=f"a{b}_{j}p{ct}")
            for d4 in range(4):
                nc.tensor.matmul(pm[:],
                                 wovt_sb[j][d4][:, ct * 128:ct * 128 + 128],
                                 Asb[d4][:],
                                 start=(d4 == 0), stop=(d4 == 3))
            tmp = sp.tile([128, T], BF16, tag=f"ptmp{ct}", name=f"a{b}_{j}pt{ct}")
            nc.scalar.activation(tmp[:], pm[:], AF.Identity,
                                 bias=bias(_bias_ov(j, ct)))
            nc.vector.scalar_tensor_tensor(
                dst[ct][:, doff:doff + T], src[ct][:, soff:soff + T], C,
                tmp[:], ALU.mult, ALU.add)

    def out_stage(b, scm):
        for tt in range(4):
            pmt = pt.tile([128, DD], BF16, tag="tr", name=f"st{b}{tt}")
            for ct in range(2):
                nc.tensor.matmul(pmt[:, ct * 128:ct * 128 + 128],
                                 scm[ct][:, tt * 128:tt * 128 + 128],
                                 identb[:], is_transpose=True)
            stg = op.tile([128, DD], F32, tag="st_sb", name=f"stsb{b}{tt}")
            nc.vector.tensor_copy(stg[:], pmt[:])
            nc.gpsimd.dma_start(
                d_states.ap()[b, tt * 128:tt * 128 + 128, :], stg[:])
        dcol = op.tile([128, 4], F32, tag="dcol", name=f"dcol{b}")
        for tt in range(4):
            pm = ps.tile([128, 321], F32, tag="mm", name=f"fc2{b}{tt}")
            for ct in range(2):
                nc.tensor.matmul(pm[:],
                                 scm[ct][:, tt * 128:tt * 128 + 128],
                                 w23t_sb[ct][:],
                                 start=(ct == 0), stop=False)
            nc.tensor.matmul(pm[:], onesb[:, 0:128],
                             b23_sb[:], start=False, stop=True)
            nc.vector.tensor_copy(dcol[:, tt:tt + 1], pm[:, 320:321])
            osb = op.tile([128, 321], F32, tag="osb", name=f"osb{b}{tt}")
            nc.scalar.activation(osb[:], pm[:], AF.Sigmoid)
            nc.gpsimd.dma_start(
                d_out.ap()[b, tt * 128:tt * 128 + 128, :], osb[:, 0:INCH])
        pmt = pt.tile([128, 128], F32, tag="tr", name=f"dn{b}")
        nc.tensor.matmul(pmt[:4, :], dcol[:], identf[:], is_transpose=True)
        done_sb = op.tile([4, 128], F32, tag="done", name=f"dnsb{b}")
        nc.scalar.activation(done_sb[:], pmt[:4, :], AF.Sigmoid)
        nc.gpsimd.dma_start(
            d_done.ap()[b].rearrange("(n p) -> n p", n=4), done_sb[:])

    # ---- interleaved batch-element pairs: the partner's matmuls fill each
    # stage's serial ACT/DVE tail so the PE never drains ----------------------
    def xtile(side, b, k):
        return [xp.tile([128, 524], BF16, tag=f"xs{side}{ct}",
                        name=f"xs{side}_{b}{k}{ct}") for ct in range(2)]

    pend_out = []          # (b, scm) of the previous pair, not yet emitted
    for pair in range(PB // 2):
        bs = (2 * pair, 2 * pair + 1)
        st = {}
        ina = {b: load_inA(b) for b in bs}
        for b in bs:
            xp1 = xtile("A", b, 1)
            st[b] = {"xp1": fc1_stage(b, xp1, ina[b])}
        if pair == 0:
            load_mid_weights()
        for b in bs:
            st[b]["kv"] = kv_stage(b)
        if pair == 0:
            load_late_weights()
        for b, scm in pend_out:
            out_stage(b, scm)
        pend_out = []
        for b in bs:
            x = st[b]
            x["xc1"] = xtile("B", b, 1)
            conv_block(0, b, x["xp1"], PADS[0], x["xc1"], 0)
        for b in bs:
            x = st[b]
            x["q1"] = attn_q2(0, b, x["xc1"], 0)
        for b in bs:
            x = st[b]
            x["ab1"] = attn_scores(0, b, x["q1"], x["kv"][0])
        for b in bs:
            x = st[b]
            x["at1"] = attn_tr(0, b, x["ab1"])
        for b in bs:
            x = st[b]
            x["xp2"] = xtile("A", b, 2)
            for ct in range(2):
                nc.vector.memset(x["xp2"][ct][:, 0:PADS[1]], 0.0)
            keysT, vnat = x["kv"]
            attn_av(0, b, x["at1"], vnat, x["xc1"], 0, x["xp2"], PADS[1])
        for b in bs:
            x = st[b]
            x["xp3"] = xtile("B", b, 2)
            for ct in range(2):
                nc.vector.memset(x["xp3"][ct][:, 0:PADS[2]], 0.0)
            conv_block(1, b, x["xp2"], PADS[1], x["xp3"], PADS[2])
        for b in bs:
            x = st[b]
            x["xp4"] = xtile("A", b, 3)
            for ct in range(2):
                nc.vector.memset(x["xp4"][ct][:, 0:PADS[3]], 0.0)
            conv_block(2, b, x["xp3"], PADS[2], x["xp4"], PADS[3])
        for b in bs:
            x = st[b]
            x["xc4"] = xtile("B", b, 3)
            conv_block(3, b, x["xp4"], PADS[3], x["xc4"], 0)
        for b in bs:
            x = st[b]
            x["q4"] = attn_q2(1, b, x["xc4"], 0)
        for b in bs:
            x = st[b]
            x["ab4"] = attn_scores(1, b, x["q4"], x["kv"][0])
        for b in bs:
            x = st[b]
            x["at4"] = attn_tr(1, b, x["ab4"])
        for b in bs:
            x = st[b]
            x["scm"] = [xp.tile([128, T], BF16, tag=f"scm{ct}",
                                name=f"scm{b}{ct}") for ct in range(2)]
            keysT, vnat = x["kv"]
            attn_av(1, b, x["at4"], vnat, x["xc4"], 0, x["scm"], 0)
            pend_out.append((b, x["scm"]))
    for b, scm in pend_out:
        out_stage(b, scm)

    ctx.close()


# --------------------------------------------------------------------------
# entry point
# --------------------------------------------------------------------------

def _get_program():
    if "nc" not in _cached:
        _cached["nc"] = _build()
    return _cached["nc"]


def run(inputs, keys, values, params, trace=False):
    nc = _get_program()
    w = _pack_weights(params)
    xin = np.asarray(inputs, dtype=np.float32).astype(BF16NP)
    keys = np.ascontiguousarray(np.asarray(keys, dtype=np.float32))
    vals = np.asarray(values, dtype=np.float32).astype(BF16NP)
    ident = np.eye(128, dtype=np.float32)
    common = dict(w1t=w["w1t"], convw=w["convw"], wqk=w["wqk"], wovt=w["wovt"],
                  w23t=w["w23t"], b23=w["b23"], biases=w["biases"],
                  identr=ident, identf=ident,
                  identb=ident.astype(BF16NP))
    in_maps = []
    for c in range(NCORES):
        sl = slice(c * PB, (c + 1) * PB)
        in_maps.append(dict(xin=xin[sl], keys=keys[sl], vals=vals[sl], **common))
    res = bass_utils.run_bass_kernel_spmd(
        nc, in_maps, core_ids=list(range(NCORES)), trace=trace)
    outs = np.concatenate([r["out"] for r in res.results], axis=0)
    states = np.concatenate([r["states"] for r in res.results], axis=0)
    done = np.concatenate([r["done"] for r in res.results], axis=0)[..., None]
    a1 = np.concatenate([np.asarray(r["align1"], dtype=np.float32)
                         for r in res.results], axis=0)
    a4 = np.concatenate([np.asarray(r["align4"], dtype=np.float32)
                         for r in res.results], axis=0)
    return (outs, states, done, a1, a4), res


def kernel(inputs, keys, values, params, prev_max_attention_idx=None):
    out, _ = run(inputs, keys, values, params)
    return out


# revision 33
# speedup vs baseline: 1.0607x; 1.0299x over previous
"""Trainium2 Bass kernel for nn_Decoder (sparse_attention).

Data-parallel over batch: B=64 split across 8 NeuronCores (8 batch elems each).
Per core, the full decoder runs with a channel-major layout ([C, T] on-chip):

  fc1 -> conv1(GLU,res) -> attn1 -> conv2..4 -> attn4 -> fc2/fc3 heads

Key layout choices:
  - x is kept channel-major [256, T=512] (2 partition tiles) through the net;
    conv taps become plain matmuls over shifted time slices of a left-padded
    SBUF buffer, attention Q/out projections read/write the same layout.
  - attn folds Wq/Wk into Wqk = Wq^T@Wk (bk drops out of softmax) and
    Wv/Wo into Wov = Wo@Wv (bv folds into the output bias via softmax sum=1),
    so keys/values are used raw: scores = (x^T Wqk) keys^T, out = Wov(attn V).
  - keys^T is produced on-chip via PE transposes (fp32r); attn^T likewise
    (bf16) to feed the attn@V matmul, which uses values in natural layout.
  - softmax skips max-subtraction (scores are in [-3, 3]) and gets row sums
    for free from the ACT Exp accum_out port.
  - dtypes: fp32r (tf32-grade, full PE speed) for the score path,
    bf16 for conv/projection weights and the x chain (SBUF capacity),
    f32 accumulation everywhere (PSUM).

Scheduling (Tile executes each engine's stream strictly in order, so emission
order is the schedule): batch elements are emitted in interleaved pairs so the
partner's matmuls cover every serial ACT/DVE tail (conv GLU, softmax);
attention is emitted in four sub-phases (q2 / scores+softmax / attn-transpose
/ A+proj) alternating between the pair so no PE instruction ever queues
behind a stalled one (head-of-line blocking); the attn@V accumulation is
st-major across all four output psums so PE consumption tracks the DVE
attn^T-evacuation rate; each pair's fc1+keys stages are hoisted before the
previous pair's output stage; weights load on the sync ring split
early/mid/late around the first input loads; outputs/aligns go out via the
otherwise-idle GpSimd SWDGE ring.

Measured on 8 axon-tunneled NeuronCores: ~733 us HW exec (728-738 over runs),
worst output rel-err 6.9e-3 (align1), PE ~96% busy within the span.
"""

import math
import numpy as np
import ml_dtypes

import concourse.bacc as bacc
import concourse.bass as bass
import concourse.tile as tile
import concourse.mybir as mybir
from concourse import bass_utils

F32 = mybir.dt.float32
F32R = mybir.dt.float32r
BF16 = mybir.dt.bfloat16
AF = mybir.ActivationFunctionType
ALU = mybir.AluOpType
BF16NP = ml_dtypes.bfloat16

NCORES = 8
B, T, S = 64, 512, 1024
PB = B // NCORES            # batch elems per core
DD, DE = 256, 512           # decoder dim / encoder dim
INCH = 320                  # fc1 in, fc2 out
KW = 5                      # conv kernel width
DILS = [1, 2, 2, 3]
PADS = [(KW - 1) * d for d in DILS]   # 4, 8, 8, 12
C = math.sqrt(0.5)
SQS = math.sqrt(S)

_cached = {}


# --------------------------------------------------------------------------
# host-side weight packing
# --------------------------------------------------------------------------

def _np(x):
    return np.asarray(x, dtype=np.float32)


def _pack_weights(params):
    p = params
    w = {}
    w["w1t"] = _np(p["fc1_W"]).T.copy().astype(BF16NP)            # [320, 256]

    convw = np.zeros((4, 2, 128, KW, 2 * DD), dtype=np.float32)
    for i in range(4):
        Wc = _np(p["convs"][i][0])                                 # [512, 256, 5]
        Ws = np.concatenate([Wc[:DD] * C, Wc[DD:]], axis=0)        # scale a-half
        convw[i] = Ws.transpose(1, 2, 0).reshape(2, 128, KW, 2 * DD)
    w["convw"] = convw.astype(BF16NP)

    wqk = np.zeros((2, 2, 128, DE), dtype=np.float32)
    wovt = np.zeros((2, 4, 128, DD), dtype=np.float32)
    bqk = np.zeros((2, DE), dtype=np.float32)
    bov = np.zeros((2, DD), dtype=np.float32)
    for j, key in enumerate(("attn1", "attn4")):
        ap = p[key]
        Wq, bq = _np(ap["Wq"]), _np(ap["bq"])
        Wk = _np(ap["Wk"])
        Wv, bv = _np(ap["Wv"]), _np(ap["bv"])
        Wo, bo = _np(ap["Wo"]), _np(ap["bo"])
        wqk[j] = (Wq.T @ Wk).reshape(2, 128, DE)
        bqk[j] = bq @ Wk
        wov = (C * SQS) * (Wo @ Wv)                                # [256, 512]
        wovt[j] = wov.T.reshape(4, 128, DD)
        bov[j] = C * (SQS * (Wo @ bv) + bo)
    w["wqk"] = wqk.astype(BF16NP)
    w["wovt"] = wovt.astype(BF16NP)

    w23 = np.concatenate([_np(p["fc2_W"]), _np(p["fc3_W"])], axis=0)  # [321, 256]
    w["w23t"] = w23.T.reshape(2, 128, 321).astype(BF16NP)
    w["b23"] = np.concatenate([_np(p["fc2_b"]), _np(p["fc3_b"])])[None, :] \
        .astype(BF16NP)                                            # [1, 321]

    # per-partition biases, packed [128, n] column-per-bias
    cols = []
    b1 = _np(p["fc1_b"])
    cols += [b1[:128], b1[128:]]                                   # 0..1
    for i in range(4):
        bc = _np(p["convs"][i][1])
        cols += [bc[:128] * C, bc[128:256] * C]                    # a-half (scaled)
        cols += [bc[256:384], bc[384:]]                            # g-half
    for j in range(2):                                             # 18..25
        for d4 in range(4):
            cols.append(bqk[j, d4 * 128:(d4 + 1) * 128])
    for j in range(2):                                             # 26..29
        for ct in range(2):
            cols.append(bov[j, ct * 128:(ct + 1) * 128])
    w["biases"] = np.stack(cols, axis=1).astype(np.float32)        # [128, 30]
    return w

BIAS_B1 = 0
def _bias_conv_a(i, h): return 2 + i * 4 + h
def _bias_conv_g(i, h): return 2 + i * 4 + 2 + h
def _bias_qk(j, d4): return 18 + j * 4 + d4
def _bias_ov(j, ct): return 26 + j * 2 + ct


# --------------------------------------------------------------------------
# device program
# --------------------------------------------------------------------------

def _build():
    nc = bacc.Bacc("TRN2", target_bir_lowering=False, debug=False,
                   enable_asserts=False, num_devices=NCORES)

    d_xin = nc.dram_tensor("xin", [PB, T, INCH], BF16, kind="ExternalInput")
    d_keys = nc.dram_tensor("keys", [PB, S, DE], BF16, kind="ExternalInput")
    d_vals = nc.dram_tensor("vals", [PB, S, DE], BF16, kind="ExternalInput")
    d_w1t = nc.dram_tensor("w1t", [INCH, DD], BF16, kind="ExternalInput")
    d_convw = nc.dram_tensor("convw", [4, 2, 128, KW, 2 * DD], BF16, kind="ExternalInput")
    d_wqk = nc.dram_tensor("wqk", [2, 2, 128, DE], BF16, kind="ExternalInput")
    d_wovt = nc.dram_tensor("wovt", [2, 4, 128, DD], BF16, kind="ExternalInput")
    d_w23t = nc.dram_tensor("w23t", [2, 128, 321], BF16, kind="ExternalInput")
    d_b23 = nc.dram_tensor("b23", [1, 321], BF16, kind="ExternalInput")
    d_biases = nc.dram_tensor("biases", [128, 30], F32, kind="ExternalInput")
    d_identr = nc.dram_tensor("identr", [128, 128], F32R, kind="ExternalInput")
    d_identf = nc.dram_tensor("identf", [128, 128], F32, kind="ExternalInput")
    d_identb = nc.dram_tensor("identb", [128, 128], BF16, kind="ExternalInput")

    d_out = nc.dram_tensor("out", [PB, T, INCH], F32, kind="ExternalOutput")
    d_states = nc.dram_tensor("states", [PB, T, DD], F32, kind="ExternalOutput")
    d_done = nc.dram_tensor("done", [PB, T], F32, kind="ExternalOutput")
    d_align = [nc.dram_tensor("align1", [PB, T, S], BF16, kind="ExternalOutput"),
               nc.dram_tensor("align4", [PB, T, S], BF16, kind="ExternalOutput")]

    with tile.TileContext(nc) as tc:
        _emit(nc, tc, d_xin, d_keys, d_vals, d_w1t, d_convw, d_wqk, d_wovt,
              d_w23t, d_b23, d_biases, d_identr, d_identf, d_identb,
              d_out, d_states, d_done, d_align)
    nc.compile()
    return nc


def _emit(nc, tc, d_xin, d_keys, d_vals, d_w1t, d_convw, d_wqk, d_wovt,
          d_w23t, d_b23, d_biases, d_identr, d_identf, d_identb,
          d_out, d_states, d_done, d_align):
    from contextlib import ExitStack
    ctx = ExitStack()
    wp = ctx.enter_context(tc.tile_pool(name="weights", bufs=1))
    xp = ctx.enter_context(tc.tile_pool(name="xchain", bufs=2))
    kp = ctx.enter_context(tc.tile_pool(name="kv", bufs=2))
    ap_ = ctx.enter_context(tc.tile_pool(name="attn", bufs=2))
    sp = ctx.enter_context(tc.tile_pool(name="small", bufs=2))
    op = ctx.enter_context(tc.tile_pool(name="outs", bufs=2))
    ps = ctx.enter_context(tc.tile_pool(name="ps", bufs=6, space="PSUM"))
    pt = ctx.enter_context(tc.tile_pool(name="pstr", bufs=2, space="PSUM"))

    # ---- persistent weights: critical few load first on the sync ring; the
    # bulk (convw[1:], attn4/fc2 weights) is emitted after pair 0's input
    # loads so it queues behind them, arriving well before first use --------
    identb = wp.tile([128, 128], BF16, tag="identb", name="identb")
    nc.sync.dma_start(identb[:], d_identb.ap())
    w1t_sb = []
    for kk in range(3):
        kw_ = 64 if kk == 2 else 128
        t_ = wp.tile([kw_, DD], BF16, tag=f"w1t{kk}", name=f"w1t{kk}")
        nc.sync.dma_start(t_[:], d_w1t.ap()[kk * 128:kk * 128 + kw_, :])
        w1t_sb.append(t_)
    bias_sb = wp.tile([128, 30], F32, tag="bias", name="bias")
    nc.sync.dma_start(bias_sb[:], d_biases.ap())
    identr = wp.tile([128, 128], F32R, tag="identr", name="identr")
    convw_sb = [[wp.tile([128, KW * 2 * DD], BF16, tag=f"cw{i}{ict}",
                         name=f"cw{i}{ict}")
                 for ict in range(2)] for i in range(4)]
    wqk_sb = [[wp.tile([128, DE], BF16, tag=f"wqk{j}{ct}", name=f"wqk{j}{ct}")
               for ct in range(2)] for j in range(2)]
    wovt_sb = [[wp.tile([128, DD], BF16, tag=f"wov{j}{d4}", name=f"wov{j}{d4}")
                for d4 in range(4)] for j in range(2)]
    w23t_sb = [wp.tile([128, 321], BF16, tag=f"w23{ct}", name=f"w23{ct}")
               for ct in range(2)]
    b23_sb = wp.tile([1, 321], BF16, tag="b23", name="b23")
    identf = wp.tile([128, 128], F32, tag="identf", name="identf")
    onesb = wp.tile([1, 128], BF16, tag="ones", name="ones")
    nc.vector.memset(onesb[:], 1.0)

    def load_mid_weights():
        nc.sync.dma_start(identr[:], d_identr.ap())
        for ict in range(2):
            nc.sync.dma_start(convw_sb[0][ict][:], d_convw.ap()[0, ict])
        for ct in range(2):
            nc.sync.dma_start(wqk_sb[0][ct][:], d_wqk.ap()[0, ct])
        for d4 in range(4):
            nc.sync.dma_start(wovt_sb[0][d4][:], d_wovt.ap()[0, d4])

    def load_late_weights():
        for i in range(1, 4):
            for ict in range(2):
                nc.sync.dma_start(convw_sb[i][ict][:], d_convw.ap()[i, ict])
        for ct in range(2):
            nc.sync.dma_start(wqk_sb[1][ct][:], d_wqk.ap()[1, ct])
        for d4 in range(4):
            nc.sync.dma_start(wovt_sb[1][d4][:], d_wovt.ap()[1, d4])
        for ct in range(2):
            nc.sync.dma_start(w23t_sb[ct][:], d_w23t.ap()[ct])
        nc.sync.dma_start(b23_sb[:], d_b23.ap())
        nc.sync.dma_start(identf[:], d_identf.ap())

    def bias(i):
        return bias_sb[:, i:i + 1]

    # ---- stages -------------------------------------------------------------
    def load_inA(b):
        inA = sp.tile([128, 4 * INCH], BF16, tag="inA", name=f"inA{b}")
        nc.sync.dma_start(
            inA[:], d_xin.ap()[b].rearrange("(tt tp) c -> tp tt c", tp=128))
        return inA

    def fc1_stage(b, xp1, inA):
        inT = []
        for cc in range(3):
            cw = 64 if cc == 2 else 128
            pmt = pt.tile([128, T], BF16, tag="tr", name=f"inT{b}{cc}")
            for tt in range(4):
                nc.tensor.matmul(
                    pmt[:cw, tt * 128:tt * 128 + 128],
                    inA[:, tt * INCH + cc * 128:tt * INCH + cc * 128 + cw],
                    identb[:], is_transpose=True)
            it = sp.tile([128, T], BF16, tag=f"inT{cc}", name=f"inTs{b}{cc}")
            nc.vector.tensor_copy(it[:cw, :], pmt[:cw, :])
            inT.append(it)
        for ct in range(2):
            pm = ps.tile([128, T], F32, tag="mm", name=f"fc1{b}{ct}")
            for kk in range(3):
                kw_ = 64 if kk == 2 else 128
                nc.tensor.matmul(pm[:],
                                 w1t_sb[kk][:kw_, ct * 128:ct * 128 + 128],
                                 inT[kk][:kw_, :],
                                 start=(kk == 0), stop=(kk == 2))
            nc.vector.memset(xp1[ct][:, 0:PADS[0]], 0.0)
            nc.scalar.activation(xp1[ct][:, PADS[0]:PADS[0] + T], pm[:], AF.Relu,
                                 bias=bias(BIAS_B1 + ct))
        return xp1

    def kv_stage(b):
        keysT = [kp.tile([128, S], BF16, tag=f"keysT{d4}", name=f"kT{b}{d4}")
                 for d4 in range(4)]
        krr = d_keys.ap()[b].rearrange("(st sp) d -> sp st d", sp=128)
        for half in range(2):
            kn2 = []
            for q in range(2):
                knat = kp.tile([128, 2 * DE], BF16, tag="knat",
                               name=f"kn{b}{half}{q}", bufs=2)
                nc.sync.dma_start(
                    knat[:], krr[:, half * 4 + q * 2:half * 4 + q * 2 + 2, :])
                kn2.append(knat)
            for d4 in range(4):
                pmt = pt.tile([128, 512], BF16, tag="tr",
                              name=f"kt{b}{half}{d4}")
                for l in range(4):
                    nc.tensor.matmul(pmt[:, l * 128:l * 128 + 128],
                                     kn2[l // 2][:, (l % 2) * DE + d4 * 128:
                                                 (l % 2) * DE + d4 * 128 + 128],
                                     identb[:], is_transpose=True)
                nc.vector.tensor_copy(keysT[d4][:, half * 512:half * 512 + 512],
                                      pmt[:])
        vnat = kp.tile([128, 8 * DE], BF16, tag="vnat", name=f"vn{b}")
        nc.sync.dma_start(
            vnat[:], d_vals.ap()[b].rearrange("(st sp) d -> sp st d", sp=128))
        return keysT, vnat

    def conv_block(i, b, src, soff, dst, doff):
        d = DILS[i]
        pms = {}
        for oct in (2, 3, 0, 1):          # g-half first: sigmoids overlap a-half
            pm = ps.tile([128, T], F32, tag="mm", name=f"cv{b}_{i}o{oct}")
            n = 0
            for ict in range(2):
                for k in range(KW):
                    nc.tensor.matmul(
                        pm[:],
                        convw_sb[i][ict][:, k * 2 * DD + oct * 128:
                                         k * 2 * DD + oct * 128 + 128],
                        src[ict][:, k * d:k * d + T],
                        start=(n == 0), stop=(n == 9))
                    n += 1
            pms[oct] = pm
        for h in range(2):
            sig = sp.tile([128, T], BF16, tag=f"sig{h}", name=f"cv{b}_{i}s{h}")
            nc.scalar.activation(sig[:], pms[2 + h][:], AF.Sigmoid,
                                 bias=bias(_bias_conv_g(i, h)))
            glu = sp.tile([128, T], BF16, tag=f"glu{h}", name=f"cv{b}_{i}g{h}")
            nc.vector.scalar_tensor_tensor(
                glu[:], pms[h][:], bias(_bias_conv_a(i, h)), sig[:],
                ALU.add, ALU.mult)
            nc.vector.scalar_tensor_tensor(
                dst[h][:, doff:doff + T], src[h][:, soff:soff + T], C, glu[:],
                ALU.mult, ALU.add)

    def attn_q2(j, b, src, soff):
        q2T = []
        for d4 in range(4):
            pm = ps.tile([128, T], F32, tag="mm", name=f"a{b}_{j}q{d4}")
            for ct in range(2):
                nc.tensor.matmul(pm[:],
                                 wqk_sb[j][ct][:, d4 * 128:d4 * 128 + 128],
                                 src[ct][:, soff:soff + T],
                                 start=(ct == 0), stop=(ct == 1))
            qt = ap_.tile([128, T], BF16, tag=f"q2T{d4}", name=f"a{b}_{j}qt{d4}")
            nc.scalar.activation(qt[:], pm[:], AF.Identity,
                                 bias=bias(_bias_qk(j, d4)))
            q2T.append(qt)
        return q2T

    def attn_scores(j, b, q2T, keysT):
        attnb = []
        for tt in range(4):
            eb = ap_.tile([128, S], BF16, tag=f"exp{tt}", name=f"a{b}_{j}e{tt}")
            zp = sp.tile([128, 2], F32, tag="zp", name=f"a{b}_{j}zp{tt}")
            for sh in range(2):
                pm = ps.tile([128, 512], F32, tag="mm", name=f"a{b}_{j}s{tt}{sh}")
                for d4 in range(4):
                    nc.tensor.matmul(pm[:],
                                     q2T[d4][:, tt * 128:tt * 128 + 128],
                                     keysT[d4][:, sh * 512:sh * 512 + 512],
                                     start=(d4 == 0), stop=(d4 == 3))
                nc.scalar.activation(eb[:, sh * 512:sh * 512 + 512], pm[:],
                                     AF.Exp, accum_out=zp[:, sh:sh + 1])
            z = sp.tile([128, 1], F32, tag="z", name=f"a{b}_{j}z{tt}")
            nc.vector.tensor_add(z[:], zp[:, 0:1], zp[:, 1:2])
            rec = sp.tile([128, 1], F32, tag="rec", name=f"a{b}_{j}r{tt}")
            nc.vector.reciprocal(rec[:], z[:])
            nc.vector.tensor_scalar_mul(eb[:], eb[:], rec[:])
            nc.gpsimd.dma_start(d_align[j].ap()[b, tt * 128:tt * 128 + 128, :],
                                eb[:])
            attnb.append(eb)
        return attnb

    def attn_tr(j, b, attnb):
        attnT = []
        for sg in range(4):
            pmt = pt.tile([128, 2 * T], BF16, tag="tr", name=f"a{b}_{j}t{sg}")
            for l in range(2):
                st = 2 * sg + l
                for tt in range(4):
                    nc.tensor.matmul(
                        pmt[:, l * T + tt * 128:l * T + tt * 128 + 128],
                        attnb[tt][:, st * 128:st * 128 + 128],
                        identb[:], is_transpose=True)
            at = ap_.tile([128, 2 * T], BF16, tag=f"attnT{sg}",
                          name=f"a{b}_{j}at{sg}")
            nc.vector.tensor_copy(at[:], pmt[:])
            attnT.append(at)
        return attnT

    def attn_av(j, b, attnT, vnat, src, soff, dst, doff):
        pmA = [ps.tile([128, T], F32, tag="mm", name=f"a{b}_{j}A{d4}")
               for d4 in range(4)]
        for st in range(8):
            for d4 in range(4):
                nc.tensor.matmul(pmA[d4][:],
                                 vnat[:, st * DE + d4 * 128:
                                      st * DE + d4 * 128 + 128],
                                 attnT[st // 2][:, (st % 2) * T:
                                                (st % 2) * T + T],
                                 start=(st == 0), stop=(st == 7))
        Asb = []
        for d4 in range(4):
            asb = ap_.tile([128, T], BF16, tag=f"Asb{d4}", name=f"a{b}_{j}as{d4}")
            nc.scalar.copy(asb[:], pmA[d4][:])
            Asb.append(asb)
        for ct in range(2):
            pm = ps.tile([128, T], F32, tag="mm", name=f"a{b}_{j}p{ct}")
            for d4 in range(4):
                nc.tensor.matmul(pm[:],
                                 wovt_sb[j][d4][:, ct * 128:ct * 128 + 128],
                                 Asb[d4][:],
                                 start=(d4 == 0), stop=(d4 == 3))
            tmp = sp.tile([128, T], BF16, tag=f"ptmp{ct}", name=f"a{b}_{j}pt{ct}")
            nc.scalar.activation(tmp[:], pm[:], AF.Identity,
                                 bias=bias(_bias_ov(j, ct)))
            nc.vector.scalar_tensor_tensor(
                dst[ct][:, doff:doff + T], src[ct][:, soff:soff + T], C,
                tmp[:], ALU.mult, ALU.add)

    def out_stage(b, scm):
        for tt in range(4):
            pmt = pt.tile([128, DD], BF16, tag="tr", name=f"st{b}{tt}")
            for ct in range(2):
                nc.tensor.matmul(pmt[:, ct * 128:ct * 128 + 128],
                                 scm[ct][:, tt * 128:tt * 128 + 128],
                                 identb[:], is_transpose=True)
            stg = op.tile([128, DD], F32, tag="st_sb", name=f"stsb{b}{tt}")
            nc.vector.tensor_copy(stg[:], pmt[:])
            nc.gpsimd.dma_start(
                d_states.ap()[b, tt * 128:tt * 128 + 128, :], stg[:])
        dcol = op.tile([128, 4], F32, tag="dcol", name=f"dcol{b}")
        for tt in range(4):
            pm = ps.tile([128, 321], F32, tag="mm", name=f"fc2{b}{tt}")
            for ct in range(2):
                nc.tensor.matmul(pm[:],
                                 scm[ct][:, tt * 128:tt * 128 + 128],
                                 w23t_sb[ct][:],
                                 start=(ct == 0), stop=False)
            nc.tensor.matmul(pm[:], onesb[:, 0:128],
                             b23_sb[:], start=False, stop=True)
            nc.vector.tensor_copy(dcol[:, tt:tt + 1], pm[:, 320:321])
            osb = op.tile([128, 321], F32, tag="osb", name=f"osb{b}{tt}")
            nc.scalar.activation(osb[:], pm[:], AF.Sigmoid)
            nc.gpsimd.dma_start(
                d_out.ap()[b, tt * 128:tt * 128 + 128, :], osb[:, 0:INCH])
        pmt = pt.tile([128, 128], F32, tag="tr", name=f"dn{b}")
        nc.tensor.matmul(pmt[:4, :], dcol[:], identf[:], is_transpose=True)
        done_sb = op.tile([4, 128], F32, tag="done", name=f"dnsb{b}")
        nc.scalar.activation(done_sb[:], pmt[:4, :], AF.Sigmoid)
        nc.gpsimd.dma_start(
            d_done.ap()[b].rearrange("(n p) -> n p", n=4), done_sb[:])

    # ---- interleaved batch-element pairs: the partner's matmuls fill each
    # stage's serial ACT/DVE tail so the PE never drains ----------------------
    def xtile(side, b, k):
        return [xp.tile([128, 524], BF16, tag=f"xs{side}{ct}",
                        name=f"xs{side}_{b}{k}{ct}") for ct in range(2)]

    pend_out = []          # (b, scm) of the previous pair, not yet emitted
    for pair in range(PB // 2):
        bs = (2 * pair, 2 * pair + 1)
        st = {}
        ina = {b: load_inA(b) for b in bs}
        for b in bs:
            xp1 = xtile("A", b, 1)
            st[b] = {"xp1": fc1_stage(b, xp1, ina[b])}
        if pair == 0:
            load_mid_weights()
        for b in bs:
            st[b]["kv"] = kv_stage(b)
        if pair == 0:
            load_late_weights()
        for b, scm in pend_out:
            out_stage(b, scm)
        pend_out = []
        for b in bs:
            x = st[b]
            x["xc1"] = xtile("B", b, 1)
            conv_block(0, b, x["xp1"], PADS[0], x["xc1"], 0)
        for b in bs:
            x = st[b]
            x["q1"] = attn_q2(0, b, x["xc1"], 0)
        for b in bs:
            x = st[b]
            x["ab1"] = attn_scores(0, b, x["q1"], x["kv"][0])
        for b in bs:
            x = st[b]
            x["at1"] = attn_tr(0, b, x["ab1"])
        for b in bs:
            x = st[b]
            x["xp2"] = xtile("A", b, 2)
            for ct in range(2):
                nc.vector.memset(x["xp2"][ct][:, 0:PADS[1]], 0.0)
            keysT, vnat = x["kv"]
            attn_av(0, b, x["at1"], vnat, x["xc1"], 0, x["xp2"], PADS[1])
        for b in bs:
            x = st[b]
            x["xp3"] = xtile("B", b, 2)
            for ct in range(2):
                nc.vector.memset(x["xp3"][ct][:, 0:PADS[2]], 0.0)
            conv_block(1, b, x["xp2"], PADS[1], x["xp3"], PADS[2])
        for b in bs:
            x = st[b]
            x["xp4"] = xtile("A", b, 3)
            for ct in range(2):
                nc.vector.memset(x["xp4"][ct][:, 0:PADS[3]], 0.0)
            conv_block(2, b, x["xp3"], PADS[2], x["xp4"], PADS[3])
        for b in bs:
            x = st[b]
            x["xc4"] = xtile("B", b, 3)
            conv_block(3, b, x["xp4"], PADS[3], x["xc4"], 0)
        for b in bs:
            x = st[b]
            x["q4"] = attn_q2(1, b, x["xc4"], 0)
        for b in bs:
            x = st[b]
            x["ab4"] = attn_scores(1, b, x["q4"], x["kv"][0])
        for b in bs:
            x = st[b]
            x["at4"] = attn_tr(1, b, x["ab4"])
        for b in bs:
            x = st[b]
            x["scm"] = [xp.tile([128, T], BF16, tag=f"scm{ct}",
                                name=f"scm{b}{ct}") for ct in range(2)]
            keysT, vnat = x["kv"]
            attn_av(1, b, x["at4"], vnat, x["xc4"], 0, x["scm"], 0)
            pend_out.append((b, x["scm"]))
    for b, scm in pend_out:
        out_stage(b, scm)

    ctx.close()


# --------------------------------------------------------------------------
# entry point
# --------------------------------------------------------------------------

def _get_program():
    if "nc" not in _cached:
        _cached["nc"] = _build()
    return _cached["nc"]


def run(inputs, keys, values, params, trace=False):
    nc = _get_program()
    w = _pack_weights(params)
    xin = np.asarray(inputs, dtype=np.float32).astype(BF16NP)
    keys = np.asarray(keys, dtype=np.float32).astype(BF16NP)
    vals = np.asarray(values, dtype=np.float32).astype(BF16NP)
    ident = np.eye(128, dtype=np.float32)
    common = dict(w1t=w["w1t"], convw=w["convw"], wqk=w["wqk"], wovt=w["wovt"],
                  w23t=w["w23t"], b23=w["b23"], biases=w["biases"],
                  identr=ident, identf=ident,
                  identb=ident.astype(BF16NP))
    in_maps = []
    for c in range(NCORES):
        sl = slice(c * PB, (c + 1) * PB)
        in_maps.append(dict(xin=xin[sl], keys=keys[sl], vals=vals[sl], **common))
    res = bass_utils.run_bass_kernel_spmd(
        nc, in_maps, core_ids=list(range(NCORES)), trace=trace)
    outs = np.concatenate([r["out"] for r in res.results], axis=0)
    states = np.concatenate([r["states"] for r in res.results], axis=0)
    done = np.concatenate([r["done"] for r in res.results], axis=0)[..., None]
    a1 = np.concatenate([np.asarray(r["align1"], dtype=np.float32)
                         for r in res.results], axis=0)
    a4 = np.concatenate([np.asarray(r["align4"], dtype=np.float32)
                         for r in res.results], axis=0)
    return (outs, states, done, a1, a4), res


def kernel(inputs, keys, values, params, prev_max_attention_idx=None):
    out, _ = run(inputs, keys, values, params)
    return out


# revision 35
# speedup vs baseline: 1.0852x; 1.0231x over previous
"""Trainium2 Bass kernel for nn_Decoder (sparse_attention).

Data-parallel over batch: B=64 split across 8 NeuronCores (8 batch elems each).
Per core, the full decoder runs with a channel-major layout ([C, T] on-chip):

  fc1 -> conv1(GLU,res) -> attn1 -> conv2..4 -> attn4 -> fc2/fc3 heads

Key layout choices:
  - x is kept channel-major [256, T=512] (2 partition tiles) through the net;
    conv taps become plain matmuls over shifted time slices of a left-padded
    SBUF buffer, attention Q/out projections read/write the same layout.
  - attn folds Wq/Wk into Wqk = Wq^T@Wk (bk drops out of softmax) and
    Wv/Wo into Wov = Wo@Wv (bv folds into the output bias via softmax sum=1),
    so keys/values are used raw: scores = (x^T Wqk) keys^T, out = Wov(attn V).
  - keys^T is produced on-chip via PE transposes; attn^T likewise, feeding
    the attn@V matmul, which uses values in natural layout.
  - softmax skips max-subtraction (scores are in [-3, 3]) and gets row sums
    for free from the ACT Exp accum_out port.
  - dtypes: bf16 operands throughout (walrus rejects mixed 32/16-bit
    matmuls, and measured end-to-end error is identical to the fp32r score
    path: the worst-case align error is set by bf16 exp storage, not logit
    precision), f32 accumulation everywhere (PSUM).

Scheduling (Tile executes each engine's stream strictly in order, so emission
order is the schedule): batch elements are emitted in interleaved pairs so the
partner's matmuls cover every serial ACT/DVE tail (conv GLU, softmax);
attention is emitted in four sub-phases (q2 / scores+softmax / attn-transpose
/ A+proj) alternating between the pair so no PE instruction ever queues
behind a stalled one (head-of-line blocking); the attn@V accumulation is
st-major across all four output psums so PE consumption tracks the DVE
attn^T-evacuation rate; each pair's fc1+keys stages are hoisted before the
previous pair's output stage; weights load on the sync ring split
early/mid/late around the first input loads; outputs/aligns go out via the
otherwise-idle GpSimd SWDGE ring.

Measured on 8 axon-tunneled NeuronCores: ~715 us HW exec (711-720 over runs),
worst output rel-err 6.9e-3 (align1), PE ~96% busy within the span.
"""

import math
import numpy as np
import ml_dtypes

import concourse.bacc as bacc
import concourse.bass as bass
import concourse.tile as tile
import concourse.mybir as mybir
from concourse import bass_utils

F32 = mybir.dt.float32
F32R = mybir.dt.float32r
BF16 = mybir.dt.bfloat16
AF = mybir.ActivationFunctionType
ALU = mybir.AluOpType
BF16NP = ml_dtypes.bfloat16

NCORES = 8
B, T, S = 64, 512, 1024
PB = B // NCORES            # batch elems per core
DD, DE = 256, 512           # decoder dim / encoder dim
INCH = 320                  # fc1 in, fc2 out
KW = 5                      # conv kernel width
DILS = [1, 2, 2, 3]
PADS = [(KW - 1) * d for d in DILS]   # 4, 8, 8, 12
C = math.sqrt(0.5)
SQS = math.sqrt(S)

_cached = {}


# --------------------------------------------------------------------------
# host-side weight packing
# --------------------------------------------------------------------------

def _np(x):
    return np.asarray(x, dtype=np.float32)


def _pack_weights(params):
    p = params
    w = {}
    w["w1t"] = _np(p["fc1_W"]).T.copy().astype(BF16NP)            # [320, 256]

    convw = np.zeros((4, 2, 128, KW, 2 * DD), dtype=np.float32)
    for i in range(4):
        Wc = _np(p["convs"][i][0])                                 # [512, 256, 5]
        Ws = np.concatenate([Wc[:DD] * C, Wc[DD:]], axis=0)        # scale a-half
        convw[i] = Ws.transpose(1, 2, 0).reshape(2, 128, KW, 2 * DD)
    w["convw"] = convw.astype(BF16NP)

    wqk = np.zeros((2, 2, 128, DE), dtype=np.float32)
    wovt = np.zeros((2, 4, 128, DD), dtype=np.float32)
    bqk = np.zeros((2, DE), dtype=np.float32)
    bov = np.zeros((2, DD), dtype=np.float32)
    for j, key in enumerate(("attn1", "attn4")):
        ap = p[key]
        Wq, bq = _np(ap["Wq"]), _np(ap["bq"])
        Wk = _np(ap["Wk"])
        Wv, bv = _np(ap["Wv"]), _np(ap["bv"])
        Wo, bo = _np(ap["Wo"]), _np(ap["bo"])
        wqk[j] = (Wq.T @ Wk).reshape(2, 128, DE)
        bqk[j] = bq @ Wk
        wov = (C * SQS) * (Wo @ Wv)                                # [256, 512]
        wovt[j] = wov.T.reshape(4, 128, DD)
        bov[j] = C * (SQS * (Wo @ bv) + bo)
    w["wqk"] = wqk.astype(BF16NP)
    w["wovt"] = wovt.astype(BF16NP)

    w23 = np.concatenate([_np(p["fc2_W"]), _np(p["fc3_W"])], axis=0)  # [321, 256]
    w["w23t"] = w23.T.reshape(2, 128, 321).astype(BF16NP)
    w["b23"] = np.concatenate([_np(p["fc2_b"]), _np(p["fc3_b"])])[None, :] \
        .astype(BF16NP)                                            # [1, 321]

    # per-partition biases, packed [128, n] column-per-bias
    cols = []
    b1 = _np(p["fc1_b"])
    cols += [b1[:128], b1[128:]]                                   # 0..1
    for i in range(4):
        bc = _np(p["convs"][i][1])
        cols += [bc[:128] * C, bc[128:256] * C]                    # a-half (scaled)
        cols += [bc[256:384], bc[384:]]                            # g-half
    for j in range(2):                                             # 18..25
        for d4 in range(4):
            cols.append(bqk[j, d4 * 128:(d4 + 1) * 128])
    for j in range(2):                                             # 26..29
        for ct in range(2):
            cols.append(bov[j, ct * 128:(ct + 1) * 128])
    w["biases"] = np.stack(cols, axis=1).astype(np.float32)        # [128, 30]
    return w

BIAS_B1 = 0
def _bias_conv_a(i, h): return 2 + i * 4 + h
def _bias_conv_g(i, h): return 2 + i * 4 + 2 + h
def _bias_qk(j, d4): return 18 + j * 4 + d4
def _bias_ov(j, ct): return 26 + j * 2 + ct


# --------------------------------------------------------------------------
# device program
# --------------------------------------------------------------------------

def _build():
    nc = bacc.Bacc("TRN2", target_bir_lowering=False, debug=False,
                   enable_asserts=False, num_devices=NCORES)

    d_xin = nc.dram_tensor("xin", [PB, T, INCH], BF16, kind="ExternalInput")
    d_keys = nc.dram_tensor("keys", [PB, S, DE], BF16, kind="ExternalInput")
    d_vals = nc.dram_tensor("vals", [PB, S, DE], BF16, kind="ExternalInput")
    d_w1t = nc.dram_tensor("w1t", [INCH, DD], BF16, kind="ExternalInput")
    d_convw = nc.dram_tensor("convw", [4, 2, 128, KW, 2 * DD], BF16, kind="ExternalInput")
    d_wqk = nc.dram_tensor("wqk", [2, 2, 128, DE], BF16, kind="ExternalInput")
    d_wovt = nc.dram_tensor("wovt", [2, 4, 128, DD], BF16, kind="ExternalInput")
    d_w23t = nc.dram_tensor("w23t", [2, 128, 321], BF16, kind="ExternalInput")
    d_b23 = nc.dram_tensor("b23", [1, 321], BF16, kind="ExternalInput")
    d_biases = nc.dram_tensor("biases", [128, 30], F32, kind="ExternalInput")
    d_identr = nc.dram_tensor("identr", [128, 128], F32R, kind="ExternalInput")
    d_identf = nc.dram_tensor("identf", [128, 128], F32, kind="ExternalInput")
    d_identb = nc.dram_tensor("identb", [128, 128], BF16, kind="ExternalInput")

    d_out = nc.dram_tensor("out", [PB, T, INCH], F32, kind="ExternalOutput")
    d_states = nc.dram_tensor("states", [PB, T, DD], F32, kind="ExternalOutput")
    d_done = nc.dram_tensor("done", [PB, T], F32, kind="ExternalOutput")
    d_align = [nc.dram_tensor("align1", [PB, T, S], BF16, kind="ExternalOutput"),
               nc.dram_tensor("align4", [PB, T, S], BF16, kind="ExternalOutput")]

    with tile.TileContext(nc) as tc:
        _emit(nc, tc, d_xin, d_keys, d_vals, d_w1t, d_convw, d_wqk, d_wovt,
              d_w23t, d_b23, d_biases, d_identr, d_identf, d_identb,
              d_out, d_states, d_done, d_align)
    nc.compile()
    return nc


def _emit(nc, tc, d_xin, d_keys, d_vals, d_w1t, d_convw, d_wqk, d_wovt,
          d_w23t, d_b23, d_biases, d_identr, d_identf, d_identb,
          d_out, d_states, d_done, d_align):
    from contextlib import ExitStack
    ctx = ExitStack()
    wp = ctx.enter_context(tc.tile_pool(name="weights", bufs=1))
    xp = ctx.enter_context(tc.tile_pool(name="xchain", bufs=2))
    kp = ctx.enter_context(tc.tile_pool(name="kv", bufs=2))
    ap_ = ctx.enter_context(tc.tile_pool(name="attn", bufs=2))
    sp = ctx.enter_context(tc.tile_pool(name="small", bufs=2))
    op = ctx.enter_context(tc.tile_pool(name="outs", bufs=2))
    ps = ctx.enter_context(tc.tile_pool(name="ps", bufs=6, space="PSUM"))
    pt = ctx.enter_context(tc.tile_pool(name="pstr", bufs=2, space="PSUM"))

    # ---- persistent weights: critical few load first on the sync ring; the
    # bulk (convw[1:], attn4/fc2 weights) is emitted after pair 0's input
    # loads so it queues behind them, arriving well before first use --------
    identb = wp.tile([128, 128], BF16, tag="identb", name="identb")
    nc.sync.dma_start(identb[:], d_identb.ap())
    w1t_sb = []
    for kk in range(3):
        kw_ = 64 if kk == 2 else 128
        t_ = wp.tile([kw_, DD], BF16, tag=f"w1t{kk}", name=f"w1t{kk}")
        nc.sync.dma_start(t_[:], d_w1t.ap()[kk * 128:kk * 128 + kw_, :])
        w1t_sb.append(t_)
    bias_sb = wp.tile([128, 30], F32, tag="bias", name="bias")
    nc.sync.dma_start(bias_sb[:], d_biases.ap())
    identr = wp.tile([128, 128], F32R, tag="identr", name="identr")
    convw_sb = [[wp.tile([128, KW * 2 * DD], BF16, tag=f"cw{i}{ict}",
                         name=f"cw{i}{ict}")
                 for ict in range(2)] for i in range(4)]
    wqk_sb = [[wp.tile([128, DE], BF16, tag=f"wqk{j}{ct}", name=f"wqk{j}{ct}")
               for ct in range(2)] for j in range(2)]
    wovt_sb = [[wp.tile([128, DD], BF16, tag=f"wov{j}{d4}", name=f"wov{j}{d4}")
                for d4 in range(4)] for j in range(2)]
    w23t_sb = [wp.tile([128, 321], BF16, tag=f"w23{ct}", name=f"w23{ct}")
               for ct in range(2)]
    b23_sb = wp.tile([1, 321], BF16, tag="b23", name="b23")
    identf = wp.tile([128, 128], F32, tag="identf", name="identf")
    onesb = wp.tile([1, 128], BF16, tag="ones", name="ones")
    nc.vector.memset(onesb[:], 1.0)

    def load_mid_weights():
        nc.sync.dma_start(identr[:], d_identr.ap())
        for ict in range(2):
            nc.sync.dma_start(convw_sb[0][ict][:], d_convw.ap()[0, ict])
        for ct in range(2):
            nc.sync.dma_start(wqk_sb[0][ct][:], d_wqk.ap()[0, ct])
        for d4 in range(4):
            nc.sync.dma_start(wovt_sb[0][d4][:], d_wovt.ap()[0, d4])

    def load_late_weights():
        for i in range(1, 4):
            for ict in range(2):
                nc.sync.dma_start(convw_sb[i][ict][:], d_convw.ap()[i, ict])
        for ct in range(2):
            nc.sync.dma_start(wqk_sb[1][ct][:], d_wqk.ap()[1, ct])
        for d4 in range(4):
            nc.sync.dma_start(wovt_sb[1][d4][:], d_wovt.ap()[1, d4])
        for ct in range(2):
            nc.sync.dma_start(w23t_sb[ct][:], d_w23t.ap()[ct])
        nc.sync.dma_start(b23_sb[:], d_b23.ap())
        nc.sync.dma_start(identf[:], d_identf.ap())

    def bias(i):
        return bias_sb[:, i:i + 1]

    # ---- stages -------------------------------------------------------------
    def load_inA(b):
        inA = sp.tile([128, 4 * INCH], BF16, tag="inA", name=f"inA{b}")
        nc.sync.dma_start(
            inA[:], d_xin.ap()[b].rearrange("(tt tp) c -> tp tt c", tp=128))
        return inA

    def fc1_stage(b, xp1, inA):
        inT = []
        for cc in range(3):
            cw = 64 if cc == 2 else 128
            pmt = pt.tile([128, T], BF16, tag="tr", name=f"inT{b}{cc}")
            for tt in range(4):
                nc.tensor.matmul(
                    pmt[:cw, tt * 128:tt * 128 + 128],
                    inA[:, tt * INCH + cc * 128:tt * INCH + cc * 128 + cw],
                    identb[:], is_transpose=True)
            it = sp.tile([128, T], BF16, tag=f"inT{cc}", name=f"inTs{b}{cc}")
            nc.vector.tensor_copy(it[:cw, :], pmt[:cw, :])
            inT.append(it)
        for ct in range(2):
            pm = ps.tile([128, T], F32, tag="mm", name=f"fc1{b}{ct}")
            for kk in range(3):
                kw_ = 64 if kk == 2 else 128
                nc.tensor.matmul(pm[:],
                                 w1t_sb[kk][:kw_, ct * 128:ct * 128 + 128],
                                 inT[kk][:kw_, :],
                                 start=(kk == 0), stop=(kk == 2))
            nc.vector.memset(xp1[ct][:, 0:PADS[0]], 0.0)
            nc.scalar.activation(xp1[ct][:, PADS[0]:PADS[0] + T], pm[:], AF.Relu,
                                 bias=bias(BIAS_B1 + ct))
        return xp1

    def kv_stage(b):
        keysT = [kp.tile([128, S], BF16, tag=f"keysT{d4}", name=f"kT{b}{d4}")
                 for d4 in range(4)]
        krr = d_keys.ap()[b].rearrange("(st sp) d -> sp st d", sp=128)
        for half in range(2):
            kn2 = []
            for q in range(2):
                knat = kp.tile([128, 2 * DE], BF16, tag="knat",
                               name=f"kn{b}{half}{q}", bufs=2)
                nc.sync.dma_start(
                    knat[:], krr[:, half * 4 + q * 2:half * 4 + q * 2 + 2, :])
                kn2.append(knat)
            for d4 in range(4):
                pmt = pt.tile([128, 512], BF16, tag="tr",
                              name=f"kt{b}{half}{d4}")
                for l in range(4):
                    nc.tensor.matmul(pmt[:, l * 128:l * 128 + 128],
                                     kn2[l // 2][:, (l % 2) * DE + d4 * 128:
                                                 (l % 2) * DE + d4 * 128 + 128],
                                     identb[:], is_transpose=True)
                nc.vector.tensor_copy(keysT[d4][:, half * 512:half * 512 + 512],
                                      pmt[:])
        vnat = kp.tile([128, 8 * DE], BF16, tag="vnat", name=f"vn{b}")
        nc.sync.dma_start(
            vnat[:], d_vals.ap()[b].rearrange("(st sp) d -> sp st d", sp=128))
        return keysT, vnat

    def conv_block(i, b, src, soff, dst, doff):
        d = DILS[i]
        pms = {}
        for oct in (2, 3, 0, 1):          # g-half first: sigmoids overlap a-half
            pm = ps.tile([128, T], F32, tag="mm", name=f"cv{b}_{i}o{oct}")
            n = 0
            for ict in range(2):
                for k in range(KW):
                    nc.tensor.matmul(
                        pm[:],
                        convw_sb[i][ict][:, k * 2 * DD + oct * 128:
                                         k * 2 * DD + oct * 128 + 128],
                        src[ict][:, k * d:k * d + T],
                        start=(n == 0), stop=(n == 9))
                    n += 1
            pms[oct] = pm
        for h in range(2):
            sig = sp.tile([128, T], BF16, tag=f"sig{h}", name=f"cv{b}_{i}s{h}")
            nc.scalar.activation(sig[:], pms[2 + h][:], AF.Sigmoid,
                                 bias=bias(_bias_conv_g(i, h)))
            glu = sp.tile([128, T], BF16, tag=f"glu{h}", name=f"cv{b}_{i}g{h}")
            nc.vector.scalar_tensor_tensor(
                glu[:], pms[h][:], bias(_bias_conv_a(i, h)), sig[:],
                ALU.add, ALU.mult)
            nc.vector.scalar_tensor_tensor(
                dst[h][:, doff:doff + T], src[h][:, soff:soff + T], C, glu[:],
                ALU.mult, ALU.add)

    def attn_q2(j, b, src, soff):
        q2T = []
        for d4 in range(4):
            pm = ps.tile([128, T], F32, tag="mm", name=f"a{b}_{j}q{d4}")
            for ct in range(2):
                nc.tensor.matmul(pm[:],
                                 wqk_sb[j][ct][:, d4 * 128:d4 * 128 + 128],
                                 src[ct][:, soff:soff + T],
                                 start=(ct == 0), stop=(ct == 1))
            qt = ap_.tile([128, T], BF16, tag=f"q2T{d4}", name=f"a{b}_{j}qt{d4}")
            nc.scalar.activation(qt[:], pm[:], AF.Identity,
                                 bias=bias(_bias_qk(j, d4)))
            q2T.append(qt)
        return q2T

    def attn_scores(j, b, q2T, keysT, last=False):
        dma_eng = nc.sync if last else nc.gpsimd
        attnb = []
        for tt in range(4):
            eb = ap_.tile([128, S], BF16, tag=f"exp{tt}", name=f"a{b}_{j}e{tt}")
            zp = sp.tile([128, 2], F32, tag="zp", name=f"a{b}_{j}zp{tt}")
            for sh in range(2):
                pm = ps.tile([128, 512], F32, tag="mm", name=f"a{b}_{j}s{tt}{sh}")
                for d4 in range(4):
                    nc.tensor.matmul(pm[:],
                                     q2T[d4][:, tt * 128:tt * 128 + 128],
                                     keysT[d4][:, sh * 512:sh * 512 + 512],
                                     start=(d4 == 0), stop=(d4 == 3))
                nc.scalar.activation(eb[:, sh * 512:sh * 512 + 512], pm[:],
                                     AF.Exp, accum_out=zp[:, sh:sh + 1])
            z = sp.tile([128, 1], F32, tag="z", name=f"a{b}_{j}z{tt}")
            nc.vector.tensor_add(z[:], zp[:, 0:1], zp[:, 1:2])
            rec = sp.tile([128, 1], F32, tag="rec", name=f"a{b}_{j}r{tt}")
            nc.vector.reciprocal(rec[:], z[:])
            nc.vector.tensor_scalar_mul(eb[:], eb[:], rec[:])
            dma_eng.dma_start(d_align[j].ap()[b, tt * 128:tt * 128 + 128, :],
                              eb[:])
            attnb.append(eb)
        return attnb

    def attn_tr(j, b, attnb):
        attnT = []
        for sg in range(4):
            pmt = pt.tile([128, 2 * T], BF16, tag="tr", name=f"a{b}_{j}t{sg}")
            for l in range(2):
                st = 2 * sg + l
                for tt in range(4):
                    nc.tensor.matmul(
                        pmt[:, l * T + tt * 128:l * T + tt * 128 + 128],
                        attnb[tt][:, st * 128:st * 128 + 128],
                        identb[:], is_transpose=True)
            at = ap_.tile([128, 2 * T], BF16, tag=f"attnT{sg}",
                          name=f"a{b}_{j}at{sg}")
            nc.vector.tensor_copy(at[:], pmt[:])
            attnT.append(at)
        return attnT

    def attn_av(j, b, attnT, vnat, src, soff, dst, doff):
        pmA = [ps.tile([128, T], F32, tag="mm", name=f"a{b}_{j}A{d4}")
               for d4 in range(4)]
        for st in range(8):
            for d4 in range(4):
                nc.tensor.matmul(pmA[d4][:],
                                 vnat[:, st * DE + d4 * 128:
                                      st * DE + d4 * 128 + 128],
                                 attnT[st // 2][:, (st % 2) * T:
                                                (st % 2) * T + T],
                                 start=(st == 0), stop=(st == 7))
        Asb = []
        for d4 in range(4):
            asb = ap_.tile([128, T], BF16, tag=f"Asb{d4}", name=f"a{b}_{j}as{d4}")
            nc.scalar.copy(asb[:], pmA[d4][:])
            Asb.append(asb)
        for ct in range(2):
            pm = ps.tile([128, T], F32, tag="mm", name=f"a{b}_{j}p{ct}")
            for d4 in range(4):
                nc.tensor.matmul(pm[:],
                                 wovt_sb[j][d4][:, ct * 128:ct * 128 + 128],
                                 Asb[d4][:],
                                 start=(d4 == 0), stop=(d4 == 3))
            tmp = sp.tile([128, T], BF16, tag=f"ptmp{ct}", name=f"a{b}_{j}pt{ct}")
            nc.scalar.activation(tmp[:], pm[:], AF.Identity,
                                 bias=bias(_bias_ov(j, ct)))
            nc.vector.scalar_tensor_tensor(
                dst[ct][:, doff:doff + T], src[ct][:, soff:soff + T], C,
                tmp[:], ALU.mult, ALU.add)

    def out_stage(b, scm, last=False):
        dma_eng = nc.sync if last else nc.gpsimd
        for tt in range(4):
            pmt = pt.tile([128, DD], BF16, tag="tr", name=f"st{b}{tt}")
            for ct in range(2):
                nc.tensor.matmul(pmt[:, ct * 128:ct * 128 + 128],
                                 scm[ct][:, tt * 128:tt * 128 + 128],
                                 identb[:], is_transpose=True)
            stg = op.tile([128, DD], F32, tag="st_sb", name=f"stsb{b}{tt}")
            nc.vector.tensor_copy(stg[:], pmt[:])
            dma_eng.dma_start(
                d_states.ap()[b, tt * 128:tt * 128 + 128, :], stg[:])
        dcol = op.tile([128, 4], F32, tag="dcol", name=f"dcol{b}")
        for tt in range(4):
            pm = ps.tile([128, 321], F32, tag="mm", name=f"fc2{b}{tt}")
            for ct in range(2):
                nc.tensor.matmul(pm[:],
                                 scm[ct][:, tt * 128:tt * 128 + 128],
                                 w23t_sb[ct][:],
                                 start=(ct == 0), stop=False)
            nc.tensor.matmul(pm[:], onesb[:, 0:128],
                             b23_sb[:], start=False, stop=True)
            nc.vector.tensor_copy(dcol[:, tt:tt + 1], pm[:, 320:321])
            osb = op.tile([128, 321], F32, tag="osb", name=f"osb{b}{tt}")
            nc.scalar.activation(osb[:], pm[:], AF.Sigmoid)
            dma_eng.dma_start(
                d_out.ap()[b, tt * 128:tt * 128 + 128, :], osb[:, 0:INCH])
        pmt = pt.tile([128, 128], F32, tag="tr", name=f"dn{b}")
        nc.tensor.matmul(pmt[:4, :], dcol[:], identf[:], is_transpose=True)
        done_sb = op.tile([4, 128], F32, tag="done", name=f"dnsb{b}")
        nc.scalar.activation(done_sb[:], pmt[:4, :], AF.Sigmoid)
        dma_eng.dma_start(
            d_done.ap()[b].rearrange("(n p) -> n p", n=4), done_sb[:])

    # ---- interleaved batch-element pairs: the partner's matmuls fill each
    # stage's serial ACT/DVE tail so the PE never drains ----------------------
    def xtile(side, b, k):
        return [xp.tile([128, 524], BF16, tag=f"xs{side}{ct}",
                        name=f"xs{side}_{b}{k}{ct}") for ct in range(2)]

    pend_out = []          # (b, scm) of the previous pair, not yet emitted
    for pair in range(PB // 2):
        bs = (2 * pair, 2 * pair + 1)
        st = {}
        ina = {b: load_inA(b) for b in bs}
        for b in bs:
            xp1 = xtile("A", b, 1)
            st[b] = {"xp1": fc1_stage(b, xp1, ina[b])}
        if pair == 0:
            load_mid_weights()
        for b in bs:
            st[b]["kv"] = kv_stage(b)
        if pair == 0:
            load_late_weights()
        for b, scm in pend_out:
            out_stage(b, scm)
        pend_out = []
        for b in bs:
            x = st[b]
            x["xc1"] = xtile("B", b, 1)
            conv_block(0, b, x["xp1"], PADS[0], x["xc1"], 0)
        for b in bs:
            x = st[b]
            x["q1"] = attn_q2(0, b, x["xc1"], 0)
        for b in bs:
            x = st[b]
            x["ab1"] = attn_scores(0, b, x["q1"], x["kv"][0])
        for b in bs:
            x = st[b]
            x["at1"] = attn_tr(0, b, x["ab1"])
        for b in bs:
            x = st[b]
            x["xp2"] = xtile("A", b, 2)
            for ct in range(2):
                nc.vector.memset(x["xp2"][ct][:, 0:PADS[1]], 0.0)
            keysT, vnat = x["kv"]
            attn_av(0, b, x["at1"], vnat, x["xc1"], 0, x["xp2"], PADS[1])
        for b in bs:
            x = st[b]
            x["xp3"] = xtile("B", b, 2)
            for ct in range(2):
                nc.vector.memset(x["xp3"][ct][:, 0:PADS[2]], 0.0)
            conv_block(1, b, x["xp2"], PADS[1], x["xp3"], PADS[2])
        for b in bs:
            x = st[b]
            x["xp4"] = xtile("A", b, 3)
            for ct in range(2):
                nc.vector.memset(x["xp4"][ct][:, 0:PADS[3]], 0.0)
            conv_block(2, b, x["xp3"], PADS[2], x["xp4"], PADS[3])
        for b in bs:
            x = st[b]
            x["xc4"] = xtile("B", b, 3)
            conv_block(3, b, x["xp4"], PADS[3], x["xc4"], 0)
        for b in bs:
            x = st[b]
            x["q4"] = attn_q2(1, b, x["xc4"], 0)
        for b in bs:
            x = st[b]
            x["ab4"] = attn_scores(1, b, x["q4"], x["kv"][0],
                                   last=(pair == PB // 2 - 1))
        for b in bs:
            x = st[b]
            x["at4"] = attn_tr(1, b, x["ab4"])
        for b in bs:
            x = st[b]
            x["scm"] = [xp.tile([128, T], BF16, tag=f"scm{ct}",
                                name=f"scm{b}{ct}") for ct in range(2)]
            keysT, vnat = x["kv"]
            attn_av(1, b, x["at4"], vnat, x["xc4"], 0, x["scm"], 0)
            pend_out.append((b, x["scm"]))
    for b, scm in pend_out:
        out_stage(b, scm, last=True)

    ctx.close()


# --------------------------------------------------------------------------
# entry point
# --------------------------------------------------------------------------

def _get_program():
    if "nc" not in _cached:
        _cached["nc"] = _build()
    return _cached["nc"]


def run(inputs, keys, values, params, trace=False):
    nc = _get_program()
    w = _pack_weights(params)
    xin = np.asarray(inputs, dtype=np.float32).astype(BF16NP)
    keys = np.asarray(keys, dtype=np.float32).astype(BF16NP)
    vals = np.asarray(values, dtype=np.float32).astype(BF16NP)
    ident = np.eye(128, dtype=np.float32)
    common = dict(w1t=w["w1t"], convw=w["convw"], wqk=w["wqk"], wovt=w["wovt"],
                  w23t=w["w23t"], b23=w["b23"], biases=w["biases"],
                  identr=ident, identf=ident,
                  identb=ident.astype(BF16NP))
    in_maps = []
    for c in range(NCORES):
        sl = slice(c * PB, (c + 1) * PB)
        in_maps.append(dict(xin=xin[sl], keys=keys[sl], vals=vals[sl], **common))
    res = bass_utils.run_bass_kernel_spmd(
        nc, in_maps, core_ids=list(range(NCORES)), trace=trace)
    outs = np.concatenate([r["out"] for r in res.results], axis=0)
    states = np.concatenate([r["states"] for r in res.results], axis=0)
    done = np.concatenate([r["done"] for r in res.results], axis=0)[..., None]
    a1 = np.concatenate([np.asarray(r["align1"], dtype=np.float32)
                         for r in res.results], axis=0)
    a4 = np.concatenate([np.asarray(r["align4"], dtype=np.float32)
                         for r in res.results], axis=0)
    return (outs, states, done, a1, a4), res


def kernel(inputs, keys, values, params, prev_max_attention_idx=None):
    out, _ = run(inputs, keys, values, params)
    return out


# revision 37
# speedup vs baseline: 1.0912x; 1.0055x over previous
"""Trainium2 Bass kernel for nn_Decoder (sparse_attention).

Data-parallel over batch: B=64 split across 8 NeuronCores (8 batch elems each).
Per core, the full decoder runs with a channel-major layout ([C, T] on-chip):

  fc1 -> conv1(GLU,res) -> attn1 -> conv2..4 -> attn4 -> fc2/fc3 heads

Key layout choices:
  - x is kept channel-major [256, T=512] (2 partition tiles) through the net;
    conv taps become plain matmuls over shifted time slices of a left-padded
    SBUF buffer, attention Q/out projections read/write the same layout.
  - attn folds Wq/Wk into Wqk = Wq^T@Wk (bk drops out of softmax) and
    Wv/Wo into Wov = Wo@Wv (bv folds into the output bias via softmax sum=1),
    so keys/values are used raw: scores = (x^T Wqk) keys^T, out = Wov(attn V).
  - keys^T is produced on-chip via PE transposes; attn^T likewise, feeding
    the attn@V matmul, which uses values in natural layout.
  - softmax skips max-subtraction (scores are in [-3, 3]) and gets row sums
    for free from the ACT Exp accum_out port.
  - dtypes: bf16 operands throughout (walrus rejects mixed 32/16-bit
    matmuls, and measured end-to-end error is identical to the fp32r score
    path: the worst-case align error is set by bf16 exp storage, not logit
    precision), f32 accumulation everywhere (PSUM).

Scheduling (Tile executes each engine's stream strictly in order, so emission
order is the schedule): batch elements are emitted in interleaved pairs so the
partner's matmuls cover every serial ACT/DVE tail (conv GLU, softmax);
attention is emitted in four sub-phases (q2 / scores+softmax / attn-transpose
/ A+proj) alternating between the pair so no PE instruction ever queues
behind a stalled one (head-of-line blocking); the attn@V accumulation is
st-major across all four output psums so PE consumption tracks the DVE
attn^T-evacuation rate; each pair's fc1+keys stages are hoisted before the
previous pair's output stage; weights load on the sync ring split
early/mid/late around the first input loads; outputs/aligns go out via the
otherwise-idle GpSimd SWDGE ring, except the final pair's, which use the
by-then-idle sync HWDGE ring so the drain tail is not paced by SWDGE latency.

Measured on 8 axon-tunneled NeuronCores: ~695 us HW exec (695-696 over runs),
worst output rel-err 6.9e-3 (align1), PE ~96% busy within the span.
"""

import math
import numpy as np
import ml_dtypes

import concourse.bacc as bacc
import concourse.bass as bass
import concourse.tile as tile
import concourse.mybir as mybir
from concourse import bass_utils

F32 = mybir.dt.float32
F32R = mybir.dt.float32r
BF16 = mybir.dt.bfloat16
AF = mybir.ActivationFunctionType
ALU = mybir.AluOpType
BF16NP = ml_dtypes.bfloat16

NCORES = 8
B, T, S = 64, 512, 1024
PB = B // NCORES            # batch elems per core
DD, DE = 256, 512           # decoder dim / encoder dim
INCH = 320                  # fc1 in, fc2 out
KW = 5                      # conv kernel width
DILS = [1, 2, 2, 3]
PADS = [(KW - 1) * d for d in DILS]   # 4, 8, 8, 12
C = math.sqrt(0.5)
SQS = math.sqrt(S)

_cached = {}


# --------------------------------------------------------------------------
# host-side weight packing
# --------------------------------------------------------------------------

def _np(x):
    return np.asarray(x, dtype=np.float32)


def _pack_weights(params):
    p = params
    w = {}
    w["w1t"] = _np(p["fc1_W"]).T.copy().astype(BF16NP)            # [320, 256]

    convw = np.zeros((4, 2, 128, KW, 2 * DD), dtype=np.float32)
    for i in range(4):
        Wc = _np(p["convs"][i][0])                                 # [512, 256, 5]
        Ws = np.concatenate([Wc[:DD] * C, Wc[DD:]], axis=0)        # scale a-half
        convw[i] = Ws.transpose(1, 2, 0).reshape(2, 128, KW, 2 * DD)
    w["convw"] = convw.astype(BF16NP)

    wqk = np.zeros((2, 2, 128, DE), dtype=np.float32)
    wovt = np.zeros((2, 4, 128, DD), dtype=np.float32)
    bqk = np.zeros((2, DE), dtype=np.float32)
    bov = np.zeros((2, DD), dtype=np.float32)
    for j, key in enumerate(("attn1", "attn4")):
        ap = p[key]
        Wq, bq = _np(ap["Wq"]), _np(ap["bq"])
        Wk = _np(ap["Wk"])
        Wv, bv = _np(ap["Wv"]), _np(ap["bv"])
        Wo, bo = _np(ap["Wo"]), _np(ap["bo"])
        wqk[j] = (Wq.T @ Wk).reshape(2, 128, DE)
        bqk[j] = bq @ Wk
        wov = (C * SQS) * (Wo @ Wv)                                # [256, 512]
        wovt[j] = wov.T.reshape(4, 128, DD)
        bov[j] = C * (SQS * (Wo @ bv) + bo)
    w["wqk"] = wqk.astype(BF16NP)
    w["wovt"] = wovt.astype(BF16NP)

    w23 = np.concatenate([_np(p["fc2_W"]), _np(p["fc3_W"])], axis=0)  # [321, 256]
    w["w23t"] = w23.T.reshape(2, 128, 321).astype(BF16NP)
    w["b23"] = np.concatenate([_np(p["fc2_b"]), _np(p["fc3_b"])])[None, :] \
        .astype(BF16NP)                                            # [1, 321]

    # per-partition biases, packed [128, n] column-per-bias
    cols = []
    b1 = _np(p["fc1_b"])
    cols += [b1[:128], b1[128:]]                                   # 0..1
    for i in range(4):
        bc = _np(p["convs"][i][1])
        cols += [bc[:128] * C, bc[128:256] * C]                    # a-half (scaled)
        cols += [bc[256:384], bc[384:]]                            # g-half
    for j in range(2):                                             # 18..25
        for d4 in range(4):
            cols.append(bqk[j, d4 * 128:(d4 + 1) * 128])
    for j in range(2):                                             # 26..29
        for ct in range(2):
            cols.append(bov[j, ct * 128:(ct + 1) * 128])
    w["biases"] = np.stack(cols, axis=1).astype(np.float32)        # [128, 30]
    return w

BIAS_B1 = 0
def _bias_conv_a(i, h): return 2 + i * 4 + h
def _bias_conv_g(i, h): return 2 + i * 4 + 2 + h
def _bias_qk(j, d4): return 18 + j * 4 + d4
def _bias_ov(j, ct): return 26 + j * 2 + ct


# --------------------------------------------------------------------------
# device program
# --------------------------------------------------------------------------

def _build():
    nc = bacc.Bacc("TRN2", target_bir_lowering=False, debug=False,
                   enable_asserts=False, num_devices=NCORES)

    d_xin = nc.dram_tensor("xin", [PB, T, INCH], BF16, kind="ExternalInput")
    d_keys = nc.dram_tensor("keys", [PB, S, DE], BF16, kind="ExternalInput")
    d_vals = nc.dram_tensor("vals", [PB, S, DE], BF16, kind="ExternalInput")
    d_w1t = nc.dram_tensor("w1t", [INCH, DD], BF16, kind="ExternalInput")
    d_convw = nc.dram_tensor("convw", [4, 2, 128, KW, 2 * DD], BF16, kind="ExternalInput")
    d_wqk = nc.dram_tensor("wqk", [2, 2, 128, DE], BF16, kind="ExternalInput")
    d_wovt = nc.dram_tensor("wovt", [2, 4, 128, DD], BF16, kind="ExternalInput")
    d_w23t = nc.dram_tensor("w23t", [2, 128, 321], BF16, kind="ExternalInput")
    d_b23 = nc.dram_tensor("b23", [1, 321], BF16, kind="ExternalInput")
    d_biases = nc.dram_tensor("biases", [128, 30], F32, kind="ExternalInput")
    d_identr = nc.dram_tensor("identr", [128, 128], F32R, kind="ExternalInput")
    d_identf = nc.dram_tensor("identf", [128, 128], F32, kind="ExternalInput")
    d_identb = nc.dram_tensor("identb", [128, 128], BF16, kind="ExternalInput")

    d_out = nc.dram_tensor("out", [PB, T, INCH], F32, kind="ExternalOutput")
    d_states = nc.dram_tensor("states", [PB, T, DD], F32, kind="ExternalOutput")
    d_done = nc.dram_tensor("done", [PB, T], F32, kind="ExternalOutput")
    d_align = [nc.dram_tensor("align1", [PB, T, S], BF16, kind="ExternalOutput"),
               nc.dram_tensor("align4", [PB, T, S], BF16, kind="ExternalOutput")]

    with tile.TileContext(nc) as tc:
        _emit(nc, tc, d_xin, d_keys, d_vals, d_w1t, d_convw, d_wqk, d_wovt,
              d_w23t, d_b23, d_biases, d_identr, d_identf, d_identb,
              d_out, d_states, d_done, d_align)
    nc.compile()
    return nc


def _emit(nc, tc, d_xin, d_keys, d_vals, d_w1t, d_convw, d_wqk, d_wovt,
          d_w23t, d_b23, d_biases, d_identr, d_identf, d_identb,
          d_out, d_states, d_done, d_align):
    from contextlib import ExitStack
    ctx = ExitStack()
    wp = ctx.enter_context(tc.tile_pool(name="weights", bufs=1))
    xp = ctx.enter_context(tc.tile_pool(name="xchain", bufs=2))
    kp = ctx.enter_context(tc.tile_pool(name="kv", bufs=2))
    ap_ = ctx.enter_context(tc.tile_pool(name="attn", bufs=2))
    sp = ctx.enter_context(tc.tile_pool(name="small", bufs=2))
    op = ctx.enter_context(tc.tile_pool(name="outs", bufs=2))
    ps = ctx.enter_context(tc.tile_pool(name="ps", bufs=6, space="PSUM"))
    pt = ctx.enter_context(tc.tile_pool(name="pstr", bufs=2, space="PSUM"))

    # ---- persistent weights: critical few load first on the sync ring; the
    # bulk (convw[1:], attn4/fc2 weights) is emitted after pair 0's input
    # loads so it queues behind them, arriving well before first use --------
    identb = wp.tile([128, 128], BF16, tag="identb", name="identb")
    nc.gpsimd.dma_start(identb[:], d_identb.ap())
    w1t_sb = []
    for kk in range(3):
        kw_ = 64 if kk == 2 else 128
        t_ = wp.tile([kw_, DD], BF16, tag=f"w1t{kk}", name=f"w1t{kk}")
        nc.gpsimd.dma_start(t_[:], d_w1t.ap()[kk * 128:kk * 128 + kw_, :])
        w1t_sb.append(t_)
    bias_sb = wp.tile([128, 30], F32, tag="bias", name="bias")
    nc.gpsimd.dma_start(bias_sb[:], d_biases.ap())
    identr = wp.tile([128, 128], F32R, tag="identr", name="identr")
    convw_sb = [[wp.tile([128, KW * 2 * DD], BF16, tag=f"cw{i}{ict}",
                         name=f"cw{i}{ict}")
                 for ict in range(2)] for i in range(4)]
    wqk_sb = [[wp.tile([128, DE], BF16, tag=f"wqk{j}{ct}", name=f"wqk{j}{ct}")
               for ct in range(2)] for j in range(2)]
    wovt_sb = [[wp.tile([128, DD], BF16, tag=f"wov{j}{d4}", name=f"wov{j}{d4}")
                for d4 in range(4)] for j in range(2)]
    w23t_sb = [wp.tile([128, 321], BF16, tag=f"w23{ct}", name=f"w23{ct}")
               for ct in range(2)]
    b23_sb = wp.tile([1, 321], BF16, tag="b23", name="b23")
    identf = wp.tile([128, 128], F32, tag="identf", name="identf")
    onesb = wp.tile([1, 128], BF16, tag="ones", name="ones")
    nc.vector.memset(onesb[:], 1.0)

    def load_mid_weights():
        nc.sync.dma_start(identr[:], d_identr.ap())
        for ict in range(2):
            nc.sync.dma_start(convw_sb[0][ict][:], d_convw.ap()[0, ict])
        for ct in range(2):
            nc.sync.dma_start(wqk_sb[0][ct][:], d_wqk.ap()[0, ct])
        for d4 in range(4):
            nc.sync.dma_start(wovt_sb[0][d4][:], d_wovt.ap()[0, d4])

    def load_late_weights():
        for i in range(1, 4):
            for ict in range(2):
                nc.sync.dma_start(convw_sb[i][ict][:], d_convw.ap()[i, ict])
        for ct in range(2):
            nc.sync.dma_start(wqk_sb[1][ct][:], d_wqk.ap()[1, ct])
        for d4 in range(4):
            nc.sync.dma_start(wovt_sb[1][d4][:], d_wovt.ap()[1, d4])
        for ct in range(2):
            nc.sync.dma_start(w23t_sb[ct][:], d_w23t.ap()[ct])
        nc.sync.dma_start(b23_sb[:], d_b23.ap())
        nc.sync.dma_start(identf[:], d_identf.ap())

    def bias(i):
        return bias_sb[:, i:i + 1]

    # ---- stages -------------------------------------------------------------
    def load_inA(b):
        inA = sp.tile([128, 4 * INCH], BF16, tag="inA", name=f"inA{b}")
        nc.sync.dma_start(
            inA[:], d_xin.ap()[b].rearrange("(tt tp) c -> tp tt c", tp=128))
        return inA

    def fc1_stage(b, xp1, inA):
        inT = []
        for cc in range(3):
            cw = 64 if cc == 2 else 128
            pmt = pt.tile([128, T], BF16, tag="tr", name=f"inT{b}{cc}")
            for tt in range(4):
                nc.tensor.matmul(
                    pmt[:cw, tt * 128:tt * 128 + 128],
                    inA[:, tt * INCH + cc * 128:tt * INCH + cc * 128 + cw],
                    identb[:], is_transpose=True)
            it = sp.tile([128, T], BF16, tag=f"inT{cc}", name=f"inTs{b}{cc}")
            nc.vector.tensor_copy(it[:cw, :], pmt[:cw, :])
            inT.append(it)
        for ct in range(2):
            pm = ps.tile([128, T], F32, tag="mm", name=f"fc1{b}{ct}")
            for kk in range(3):
                kw_ = 64 if kk == 2 else 128
                nc.tensor.matmul(pm[:],
                                 w1t_sb[kk][:kw_, ct * 128:ct * 128 + 128],
                                 inT[kk][:kw_, :],
                                 start=(kk == 0), stop=(kk == 2))
            nc.vector.memset(xp1[ct][:, 0:PADS[0]], 0.0)
            nc.scalar.activation(xp1[ct][:, PADS[0]:PADS[0] + T], pm[:], AF.Relu,
                                 bias=bias(BIAS_B1 + ct))
        return xp1

    def kv_stage(b):
        keysT = [kp.tile([128, S], BF16, tag=f"keysT{d4}", name=f"kT{b}{d4}")
                 for d4 in range(4)]
        krr = d_keys.ap()[b].rearrange("(st sp) d -> sp st d", sp=128)
        for half in range(2):
            kn2 = []
            for q in range(2):
                knat = kp.tile([128, 2 * DE], BF16, tag="knat",
                               name=f"kn{b}{half}{q}", bufs=4)
                nc.sync.dma_start(
                    knat[:], krr[:, half * 4 + q * 2:half * 4 + q * 2 + 2, :])
                kn2.append(knat)
            for d4 in range(4):
                pmt = pt.tile([128, 512], BF16, tag="tr",
                              name=f"kt{b}{half}{d4}")
                for l in range(4):
                    nc.tensor.matmul(pmt[:, l * 128:l * 128 + 128],
                                     kn2[l // 2][:, (l % 2) * DE + d4 * 128:
                                                 (l % 2) * DE + d4 * 128 + 128],
                                     identb[:], is_transpose=True)
                nc.vector.tensor_copy(keysT[d4][:, half * 512:half * 512 + 512],
                                      pmt[:])
        vnat = kp.tile([128, 8 * DE], BF16, tag="vnat", name=f"vn{b}")
        nc.sync.dma_start(
            vnat[:], d_vals.ap()[b].rearrange("(st sp) d -> sp st d", sp=128))
        return keysT, vnat

    def conv_block(i, b, src, soff, dst, doff):
        d = DILS[i]
        pms = {}
        for oct in (2, 3, 0, 1):          # g-half first: sigmoids overlap a-half
            pm = ps.tile([128, T], F32, tag="mm", name=f"cv{b}_{i}o{oct}")
            n = 0
            for ict in range(2):
                for k in range(KW):
                    nc.tensor.matmul(
                        pm[:],
                        convw_sb[i][ict][:, k * 2 * DD + oct * 128:
                                         k * 2 * DD + oct * 128 + 128],
                        src[ict][:, k * d:k * d + T],
                        start=(n == 0), stop=(n == 9))
                    n += 1
            pms[oct] = pm
        for h in range(2):
            sig = sp.tile([128, T], BF16, tag=f"sig{h}", name=f"cv{b}_{i}s{h}")
            nc.scalar.activation(sig[:], pms[2 + h][:], AF.Sigmoid,
                                 bias=bias(_bias_conv_g(i, h)))
            glu = sp.tile([128, T], BF16, tag=f"glu{h}", name=f"cv{b}_{i}g{h}")
            nc.vector.scalar_tensor_tensor(
                glu[:], pms[h][:], bias(_bias_conv_a(i, h)), sig[:],
                ALU.add, ALU.mult)
            nc.vector.scalar_tensor_tensor(
                dst[h][:, doff:doff + T], src[h][:, soff:soff + T], C, glu[:],
                ALU.mult, ALU.add)

    def attn_q2(j, b, src, soff):
        q2T = []
        for d4 in range(4):
            pm = ps.tile([128, T], F32, tag="mm", name=f"a{b}_{j}q{d4}")
            for ct in range(2):
                nc.tensor.matmul(pm[:],
                                 wqk_sb[j][ct][:, d4 * 128:d4 * 128 + 128],
                                 src[ct][:, soff:soff + T],
                                 start=(ct == 0), stop=(ct == 1))
            qt = ap_.tile([128, T], BF16, tag=f"q2T{d4}", name=f"a{b}_{j}qt{d4}")
            nc.scalar.activation(qt[:], pm[:], AF.Identity,
                                 bias=bias(_bias_qk(j, d4)))
            q2T.append(qt)
        return q2T

    def attn_scores(j, b, q2T, keysT, last=False):
        dma_eng = nc.sync if last else nc.gpsimd
        attnb = []
        for tt in range(4):
            eb = ap_.tile([128, S], BF16, tag=f"exp{tt}", name=f"a{b}_{j}e{tt}")
            zp = sp.tile([128, 2], F32, tag="zp", name=f"a{b}_{j}zp{tt}")
            for sh in range(2):
                pm = ps.tile([128, 512], F32, tag="mm", name=f"a{b}_{j}s{tt}{sh}")
                for d4 in range(4):
                    nc.tensor.matmul(pm[:],
                                     q2T[d4][:, tt * 128:tt * 128 + 128],
                                     keysT[d4][:, sh * 512:sh * 512 + 512],
                                     start=(d4 == 0), stop=(d4 == 3))
                nc.scalar.activation(eb[:, sh * 512:sh * 512 + 512], pm[:],
                                     AF.Exp, accum_out=zp[:, sh:sh + 1])
            z = sp.tile([128, 1], F32, tag="z", name=f"a{b}_{j}z{tt}")
            nc.vector.tensor_add(z[:], zp[:, 0:1], zp[:, 1:2])
            rec = sp.tile([128, 1], F32, tag="rec", name=f"a{b}_{j}r{tt}")
            nc.vector.reciprocal(rec[:], z[:])
            nc.vector.tensor_scalar_mul(eb[:], eb[:], rec[:])
            dma_eng.dma_start(d_align[j].ap()[b, tt * 128:tt * 128 + 128, :],
                              eb[:])
            attnb.append(eb)
        return attnb

    def attn_tr(j, b, attnb):
        attnT = []
        for sg in range(4):
            pmt = pt.tile([128, 2 * T], BF16, tag="tr", name=f"a{b}_{j}t{sg}")
            for l in range(2):
                st = 2 * sg + l
                for tt in range(4):
                    nc.tensor.matmul(
                        pmt[:, l * T + tt * 128:l * T + tt * 128 + 128],
                        attnb[tt][:, st * 128:st * 128 + 128],
                        identb[:], is_transpose=True)
            at = ap_.tile([128, 2 * T], BF16, tag=f"attnT{sg}",
                          name=f"a{b}_{j}at{sg}")
            nc.vector.tensor_copy(at[:], pmt[:])
            attnT.append(at)
        return attnT

    def attn_av(j, b, attnT, vnat, src, soff, dst, doff):
        pmA = [ps.tile([128, T], F32, tag="mm", name=f"a{b}_{j}A{d4}")
               for d4 in range(4)]
        for st in range(8):
            for d4 in range(4):
                nc.tensor.matmul(pmA[d4][:],
                                 vnat[:, st * DE + d4 * 128:
                                      st * DE + d4 * 128 + 128],
                                 attnT[st // 2][:, (st % 2) * T:
                                                (st % 2) * T + T],
                                 start=(st == 0), stop=(st == 7))
        Asb = []
        for d4 in range(4):
            asb = ap_.tile([128, T], BF16, tag=f"Asb{d4}", name=f"a{b}_{j}as{d4}")
            nc.scalar.copy(asb[:], pmA[d4][:])
            Asb.append(asb)
        for ct in range(2):
            pm = ps.tile([128, T], F32, tag="mm", name=f"a{b}_{j}p{ct}")
            for d4 in range(4):
                nc.tensor.matmul(pm[:],
                                 wovt_sb[j][d4][:, ct * 128:ct * 128 + 128],
                                 Asb[d4][:],
                                 start=(d4 == 0), stop=(d4 == 3))
            tmp = sp.tile([128, T], BF16, tag=f"ptmp{ct}", name=f"a{b}_{j}pt{ct}")
            nc.scalar.activation(tmp[:], pm[:], AF.Identity,
                                 bias=bias(_bias_ov(j, ct)))
            nc.vector.scalar_tensor_tensor(
                dst[ct][:, doff:doff + T], src[ct][:, soff:soff + T], C,
                tmp[:], ALU.mult, ALU.add)

    def out_stage(b, scm, last=False):
        dma_eng = nc.sync if last else nc.gpsimd
        for tt in range(4):
            pmt = pt.tile([128, DD], BF16, tag="tr", name=f"st{b}{tt}")
            for ct in range(2):
                nc.tensor.matmul(pmt[:, ct * 128:ct * 128 + 128],
                                 scm[ct][:, tt * 128:tt * 128 + 128],
                                 identb[:], is_transpose=True)
            stg = op.tile([128, DD], F32, tag="st_sb", name=f"stsb{b}{tt}")
            nc.vector.tensor_copy(stg[:], pmt[:])
            dma_eng.dma_start(
                d_states.ap()[b, tt * 128:tt * 128 + 128, :], stg[:])
        dcol = op.tile([128, 4], F32, tag="dcol", name=f"dcol{b}")
        for tt in range(4):
            pm = ps.tile([128, 321], F32, tag="mm", name=f"fc2{b}{tt}")
            for ct in range(2):
                nc.tensor.matmul(pm[:],
                                 scm[ct][:, tt * 128:tt * 128 + 128],
                                 w23t_sb[ct][:],
                                 start=(ct == 0), stop=False)
            nc.tensor.matmul(pm[:], onesb[:, 0:128],
                             b23_sb[:], start=False, stop=True)
            nc.vector.tensor_copy(dcol[:, tt:tt + 1], pm[:, 320:321])
            osb = op.tile([128, 321], F32, tag="osb", name=f"osb{b}{tt}")
            nc.scalar.activation(osb[:], pm[:], AF.Sigmoid)
            dma_eng.dma_start(
                d_out.ap()[b, tt * 128:tt * 128 + 128, :], osb[:, 0:INCH])
        pmt = pt.tile([128, 128], F32, tag="tr", name=f"dn{b}")
        nc.tensor.matmul(pmt[:4, :], dcol[:], identf[:], is_transpose=True)
        done_sb = op.tile([4, 128], F32, tag="done", name=f"dnsb{b}")
        nc.scalar.activation(done_sb[:], pmt[:4, :], AF.Sigmoid)
        dma_eng.dma_start(
            d_done.ap()[b].rearrange("(n p) -> n p", n=4), done_sb[:])

    # ---- interleaved batch-element pairs: the partner's matmuls fill each
    # stage's serial ACT/DVE tail so the PE never drains ----------------------
    def xtile(side, b, k):
        return [xp.tile([128, 524], BF16, tag=f"xs{side}{ct}",
                        name=f"xs{side}_{b}{k}{ct}") for ct in range(2)]

    pend_out = []          # (b, scm) of the previous pair, not yet emitted
    for pair in range(PB // 2):
        bs = (2 * pair, 2 * pair + 1)
        st = {}
        ina = {b: load_inA(b) for b in bs}
        for b in bs:
            xp1 = xtile("A", b, 1)
            st[b] = {"xp1": fc1_stage(b, xp1, ina[b])}
        if pair == 0:
            load_mid_weights()
        for b in bs:
            st[b]["kv"] = kv_stage(b)
        if pair == 0:
            load_late_weights()
        for b, scm in pend_out:
            out_stage(b, scm)
        pend_out = []
        for b in bs:
            x = st[b]
            x["xc1"] = xtile("B", b, 1)
            conv_block(0, b, x["xp1"], PADS[0], x["xc1"], 0)
        for b in bs:
            x = st[b]
            x["q1"] = attn_q2(0, b, x["xc1"], 0)
        for b in bs:
            x = st[b]
            x["ab1"] = attn_scores(0, b, x["q1"], x["kv"][0])
        for b in bs:
            x = st[b]
            x["at1"] = attn_tr(0, b, x["ab1"])
        for b in bs:
            x = st[b]
            x["xp2"] = xtile("A", b, 2)
            for ct in range(2):
                nc.vector.memset(x["xp2"][ct][:, 0:PADS[1]], 0.0)
            keysT, vnat = x["kv"]
            attn_av(0, b, x["at1"], vnat, x["xc1"], 0, x["xp2"], PADS[1])
        for b in bs:
            x = st[b]
            x["xp3"] = xtile("B", b, 2)
            for ct in range(2):
                nc.vector.memset(x["xp3"][ct][:, 0:PADS[2]], 0.0)
            conv_block(1, b, x["xp2"], PADS[1], x["xp3"], PADS[2])
        for b in bs:
            x = st[b]
            x["xp4"] = xtile("A", b, 3)
            for ct in range(2):
                nc.vector.memset(x["xp4"][ct][:, 0:PADS[3]], 0.0)
            conv_block(2, b, x["xp3"], PADS[2], x["xp4"], PADS[3])
        for b in bs:
            x = st[b]
            x["xc4"] = xtile("B", b, 3)
            conv_block(3, b, x["xp4"], PADS[3], x["xc4"], 0)
        for b in bs:
            x = st[b]
            x["q4"] = attn_q2(1, b, x["xc4"], 0)
        for b in bs:
            x = st[b]
            x["ab4"] = attn_scores(1, b, x["q4"], x["kv"][0],
                                   last=(pair == PB // 2 - 1))
        for b in bs:
            x = st[b]
            x["at4"] = attn_tr(1, b, x["ab4"])
        for b in bs:
            x = st[b]
            x["scm"] = [xp.tile([128, T], BF16, tag=f"scm{ct}",
                                name=f"scm{b}{ct}") for ct in range(2)]
            keysT, vnat = x["kv"]
            attn_av(1, b, x["at4"], vnat, x["xc4"], 0, x["scm"], 0)
            pend_out.append((b, x["scm"]))
    for b, scm in pend_out:
        out_stage(b, scm, last=True)

    ctx.close()


# --------------------------------------------------------------------------
# entry point
# --------------------------------------------------------------------------

def _get_program():
    if "nc" not in _cached:
        _cached["nc"] = _build()
    return _cached["nc"]


def run(inputs, keys, values, params, trace=False):
    nc = _get_program()
    w = _pack_weights(params)
    xin = np.asarray(inputs, dtype=np.float32).astype(BF16NP)
    keys = np.asarray(keys, dtype=np.float32).astype(BF16NP)
    vals = np.asarray(values, dtype=np.float32).astype(BF16NP)
    ident = np.eye(128, dtype=np.float32)
    common = dict(w1t=w["w1t"], convw=w["convw"], wqk=w["wqk"], wovt=w["wovt"],
                  w23t=w["w23t"], b23=w["b23"], biases=w["biases"],
                  identr=ident, identf=ident,
                  identb=ident.astype(BF16NP))
    in_maps = []
    for c in range(NCORES):
        sl = slice(c * PB, (c + 1) * PB)
        in_maps.append(dict(xin=xin[sl], keys=keys[sl], vals=vals[sl], **common))
    res = bass_utils.run_bass_kernel_spmd(
        nc, in_maps, core_ids=list(range(NCORES)), trace=trace)
    outs = np.concatenate([r["out"] for r in res.results], axis=0)
    states = np.concatenate([r["states"] for r in res.results], axis=0)
    done = np.concatenate([r["done"] for r in res.results], axis=0)[..., None]
    a1 = np.concatenate([np.asarray(r["align1"], dtype=np.float32)
                         for r in res.results], axis=0)
    a4 = np.concatenate([np.asarray(r["align4"], dtype=np.float32)
                         for r in res.results], axis=0)
    return (outs, states, done, a1, a4), res


def kernel(inputs, keys, values, params, prev_max_attention_idx=None):
    out, _ = run(inputs, keys, values, params)
    return out
